# revision 1
# baseline (speedup 1.0000x reference)
"""Trainium2 Bass kernel for nn_Difference_Module (dense transformer block).

Math (per batch, N=4096, D=64, H=256):
    S      = q @ k^T / 8                       [N, N]
    attn   = softmax(S) @ v                    [N, D]
    v1     = (v - attn) @ W_dif + b_dif        [N, D]
    v_new  = S @ v1 + q
    h      = layernorm(v_new) * gamma + beta
    out    = gelu(h @ W1 + b1) @ W2 + b2 + v_new

Key algebraic optimization: S is rank-64 (S = q @ k^T / 8), so
    S @ v1 = q @ (k^T @ v1) / 8
which removes any need to materialize or recompute S for the second use.
Only the softmax path touches the full [N, N] score matrix, flash-style:
we compute S^T tiles (k-index on partitions, q-index on the free axis),
exponentiate without max-subtraction (scores ~ N(0,1), no overflow), and
accumulate exp(S)^T-weighted V with an appended ones-column to get the
softmax denominators in the same matmul.

The scalar (ACT) engine is the bottleneck: 16.8M exps at ~1 elem/lane/
cycle. Everything else is arranged to hide under it:
  - exp runs in [128, 1536] units (amortizes the ~352-cycle per-
    instruction overhead) out of a manually rotated 6-bank PSUM region.
  - all transposes are regular matmuls against an identity (pipelines at
    ~85ns/tile vs ~350ns for transpose-mode), mostly bf16.
  - pass-2 rstd = exp(-0.5*ln(var+eps)) stays in the natural_log_exp
    ACT table set (pinned by a dummy ln at init), so the only table
    switch in the whole kernel is the one load for gelu.
  - softmax denominators use the fast Newton-Raphson reciprocal.

Sharding: pure data parallel, one batch per NeuronCore (B=8, 8 cores),
no collectives.
"""

import sys
from contextlib import ExitStack

import numpy as np

for _p in ("/opt/trn_rl_repo",):
    if _p not in sys.path:
        sys.path.insert(0, _p)

import concourse.bass as bass
import concourse.bacc as bacc
import concourse.tile as tile
from concourse import mybir
from concourse.bass_utils import run_bass_kernel_spmd
from concourse.masks import make_identity

N = 4096          # sequence length per batch
D = 64            # model dim
H = 256           # mlp hidden dim
B = 8             # batches == cores
P = 128           # SBUF partitions
NT = N // P       # 32 row-tiles of 128
CH = 512          # chunk of the q/free axis
NCH = N // CH     # 8 chunks
TPC = CH // P     # 4 row-tiles per chunk
JT2 = NT // 2     # 16 QK iterations per chunk (2 j-tiles each)
G = NCH * JT2     # 128 global QK iterations
RB = 3            # QK psum region holds 3 iterations (3 * 1024 f32 = 6 banks)
EXPU = 1536       # exp unit: half the region
EPS = 1e-5
SCALE = 0.125     # 1/sqrt(D)

F32 = mybir.dt.float32
F32R = mybir.dt.float32r
BF16 = mybir.dt.bfloat16
FP8 = mybir.dt.float8e4
ALU = mybir.AluOpType
ACTF = mybir.ActivationFunctionType


def build_nc() -> bass.Bass:
    nc = bacc.Bacc("TRN2", target_bir_lowering=False, debug=False, num_devices=B)

    q = nc.dram_tensor("q", [N, D], F32, kind="ExternalInput").ap()
    k = nc.dram_tensor("k", [N, D], F32, kind="ExternalInput").ap()
    v = nc.dram_tensor("v", [N, D], F32, kind="ExternalInput").ap()
    w_dif = nc.dram_tensor("W_dif", [D, D], F32, kind="ExternalInput").ap()
    b_dif = nc.dram_tensor("b_dif", [D], F32, kind="ExternalInput").ap()
    gamma = nc.dram_tensor("gamma", [D], F32, kind="ExternalInput").ap()
    beta = nc.dram_tensor("beta", [D], F32, kind="ExternalInput").ap()
    w1 = nc.dram_tensor("W1", [D, H], F32, kind="ExternalInput").ap()
    b1 = nc.dram_tensor("b1", [H], F32, kind="ExternalInput").ap()
    w2 = nc.dram_tensor("W2", [H, D], F32, kind="ExternalInput").ap()
    b2 = nc.dram_tensor("b2", [D], F32, kind="ExternalInput").ap()
    out = nc.dram_tensor("out", [N, D], F32, kind="ExternalOutput").ap()

    with tile.TileContext(nc) as tc:
        with ExitStack() as ctx:
            _body(ctx, tc, q, k, v, w_dif, b_dif, gamma, beta, w1, b1, w2, b2, out)
    nc.compile()
    return nc


def _bcast_free(nc, dst, src_dram):
    """DMA a [D] dram vector into dst [P, reps, D]: broadcast on partitions,
    replicated `reps` times along the free axis."""
    reps = dst.shape[1]
    for i in range(reps):
        nc.sync.dma_start(
            out=dst[:, i, :],
            in_=bass.AP(
                tensor=src_dram.tensor,
                offset=src_dram.offset,
                ap=[[0, P]] + src_dram.ap,
            ),
        )


def _body(ctx, tc, q, k, v, w_dif, b_dif, gamma, beta, w1, b1, w2, b2, out):
    nc = tc.nc

    consts = ctx.enter_context(tc.tile_pool(name="consts", bufs=1))
    big = ctx.enter_context(tc.tile_pool(name="big", bufs=1))
    work = ctx.enter_context(tc.tile_pool(name="work", bufs=4))

    # ---------------- DMA loads (k first: it gates everything) ----------
    k_nat = big.tile([P, NT, D], F32, tag="k_nat")
    q_nat = big.tile([P, NT, D], F32, tag="q_nat")
    v_nat = big.tile([P, NT, D], F32, tag="v_nat")
    GBD = 8
    k_rr = k.rearrange("(t p) d -> p t d", p=P)
    q_rr = q.rearrange("(t p) d -> p t d", p=P)
    v_rr = v.rearrange("(t p) d -> p t d", p=P)
    for g in range(NT // GBD):
        nc.sync.dma_start(out=k_nat[:, g * GBD:(g + 1) * GBD, :],
                          in_=k_rr[:, g * GBD:(g + 1) * GBD, :])
    nc.sync.dma_start(out=q_nat[:, 0:GBD, :], in_=q_rr[:, 0:GBD, :])
    for g in range(NT // GBD):
        nc.sync.dma_start(out=v_nat[:, g * GBD:(g + 1) * GBD, :],
                          in_=v_rr[:, g * GBD:(g + 1) * GBD, :])
    for g in range(1, NT // GBD):
        nc.sync.dma_start(out=q_nat[:, g * GBD:(g + 1) * GBD, :],
                          in_=q_rr[:, g * GBD:(g + 1) * GBD, :])

    # ---------------- constants / parameters ----------------
    ident = consts.tile([P, P], F32, tag="ident")
    make_identity(nc, ident)
    ident_bf = consts.tile([P, P], BF16, tag="ident_bf")
    nc.vector.tensor_copy(ident_bf, ident)

    wdif_f = consts.tile([D, D], F32, tag="wdif_f")
    nc.sync.dma_start(out=wdif_f, in_=w_dif)
    wdif_sb = consts.tile([D, D], BF16, tag="wdif")
    nc.vector.tensor_copy(wdif_sb, wdif_f)

    w1_sb = consts.tile([D, H], F32, tag="w1")
    nc.sync.dma_start(out=w1_sb, in_=w1)
    gamma_sb = consts.tile([D, 1], F32, tag="gamma")
    nc.sync.dma_start(out=gamma_sb, in_=gamma[:, None])
    beta_sb = consts.tile([D, 1], F32, tag="beta")
    nc.sync.dma_start(out=beta_sb, in_=beta[:, None])

    # Fold LN gamma into W1 (h_hat * gamma @ W1 = h_hat @ (gamma[:,None]*W1));
    # beta's contribution lands in the bias: b1' = b1 + beta @ W1.
    w1p_sb = consts.tile([D, H], BF16, tag="w1p")
    nc.vector.tensor_scalar_mul(w1p_sb, w1_sb, gamma_sb)

    b1_sb = consts.tile([P, 2], F32, tag="b1")
    nc.sync.dma_start(out=b1_sb, in_=b1.rearrange("(a p) -> p a", p=P))

    w2f_sb = consts.tile([P, 2, D], F32, tag="w2f")
    nc.sync.dma_start(out=w2f_sb, in_=w2.rearrange("(a p) d -> p a d", p=P))
    w2_sb = consts.tile([P, 2, D], BF16, tag="w2")
    nc.vector.tensor_copy(w2_sb, w2f_sb)

    b2_bc = consts.tile([P, TPC, D], F32, tag="b2bc")
    _bcast_free(nc, b2_bc, b2)
    bdif_bc = consts.tile([P, TPC, D], F32, tag="bdifbc")
    _bcast_free(nc, bdif_bc, b_dif)

    ones_sb = consts.tile([1, D], F32, tag="ones")
    nc.vector.memset(ones_sb, 1.0)
    eps_sb = consts.tile([P, 1], F32, tag="eps")
    nc.vector.memset(eps_sb, EPS)
    nbias_sb = consts.tile([P, 1], F32, tag="nbias")
    nc.vector.memset(nbias_sb, -2.5)


    # ---------------- bf16 copies + transposed layouts ----------------
    k_bf = big.tile([P, NT, D], BF16, tag="k_bf")
    q_bf = big.tile([P, NT, D], BF16, tag="q_bf")

    qT = big.tile([P, N], BF16, tag="qT")   # rows 0-63 and 64-127 both hold q^T
    kT = big.tile([P, N], BF16, tag="kT")   # rows 0-63 and 64-127 both hold k^T
    vT = big.tile([D, N], BF16, tag="vT")

    b1p_sb = consts.tile([P, 2], F32, tag="b1p")

    # init-phase PSUM pool: b1p matmuls + k transposes + q chunk-0 transpose.
    # Closed before the pass-1 pools open so the banks get reused.
    with ExitStack() as sctx:
        ps_init = sctx.enter_context(tc.tile_pool(name="ps_init", bufs=2, space="PSUM"))
        for a in range(2):
            bw = ps_init.tile([P, 1], F32, tag="bw")
            nc.tensor.matmul(
                bw, w1_sb[:, a * P:(a + 1) * P], beta_sb, start=True, stop=True
            )
            nc.vector.tensor_add(b1p_sb[:, a:a + 1], bw, b1_sb[:, a:a + 1])

        # k transposes: 4 groups of 8 tiles (cast -> matmul-transpose -> evac,
        # pipelined per DMA piece), regular matmuls vs identity
        for gidx in range(4):
            nc.vector.tensor_copy(k_bf[:, gidx * 8:(gidx + 1) * 8, :],
                                  k_nat[:, gidx * 8:(gidx + 1) * 8, :])
            pt = ps_init.tile([D, 8 * P], F32, tag="ktr", name=f"ktr{gidx}")
            for s in range(8):
                t = gidx * 8 + s
                nc.tensor.matmul(pt[:, s * P:(s + 1) * P], k_bf[:, t, :], ident_bf,
                                 start=True, stop=True)
            nc.vector.tensor_copy(kT[0:D, gidx * 8 * P:(gidx + 1) * 8 * P], pt)
        # duplicate k^T into rows 64..127 on the idle gpsimd engine
        nc.gpsimd.tensor_copy(kT[D:P, :], kT[0:D, :])
        nc.vector.tensor_copy(q_bf[:, 0:TPC, :], q_nat[:, 0:TPC, :])

        # q chunk 0 transpose (4 tiles)
        pt = ps_init.tile([D, TPC * P], F32, tag="qtr0")
        for s in range(TPC):
            nc.tensor.matmul(pt[:, s * P:(s + 1) * P], q_bf[:, s, :], ident_bf,
                             start=True, stop=True)
        nc.vector.tensor_copy(qT[0:D, 0:CH], pt)
        nc.gpsimd.tensor_copy(qT[D:P, 0:CH], qT[0:D, 0:CH])

    # V with an appended ones column: the PV matmul then also produces the
    # softmax denominators (row 64 of the accumulator).
    # DoubleRow fp8 layout: pairs of j-tiles interleaved on the ko axis,
    # inner stride padded to 80 bytes (16-aligned). Ones column -> denominators.
    v_aug = big.tile([P, NT // 2, 2, 80], FP8, tag="v_aug")
    with nc.allow_low_precision(reason="softmax-averaged fp8 PV"):
        for gidx in range(4):
            nc.vector.tensor_copy(v_aug[:, gidx * 4:(gidx + 1) * 4, :, 0:D],
                                  v_nat[:, gidx * 8:(gidx + 1) * 8, :])
    nc.vector.memset(v_aug[:, :, :, D:D + 1], 1.0)

    v1_nat = big.tile([P, NT, D], BF16, tag="v1_nat")
    T_sb = big.tile([D, D], F32, tag="T_sb")
    nc.vector.memset(T_sb, 0.0)
    pT = big.tile([P, 2 * RB, CH], FP8, tag="pT")   # rotating exp output

    # ---------------- pass 1: flash attention + dif_proj + T ----------------
    with ExitStack() as p1:
        ps_qk_pool = p1.enter_context(tc.tile_pool(name="ps_qk", bufs=1, space="PSUM"))
        ps_attn = p1.enter_context(tc.tile_pool(name="ps_attn", bufs=1, space="PSUM"))
        ps_tail_pool = p1.enter_context(tc.tile_pool(name="ps_tail", bufs=1, space="PSUM"))

        qk_ps = ps_qk_pool.tile([P, RB * 2 * CH], F32, tag="qk")    # 6 banks
        tail_ps = ps_tail_pool.tile([P, CH], F32, tag="tail")       # 1 bank

        attn_tiles = {}
        chunk_state = {}

        def emit_qk(g):
            c, jt2 = divmod(g, JT2)
            r = g % RB
            st = qk_ps[:, r * 2 * CH:(r + 1) * 2 * CH]
            i0 = c * CH
            for s in range(2):
                jt = jt2 * 2 + s
                r0 = s * D
                nc.tensor.matmul(
                    st[:, s * CH:(s + 1) * CH],
                    kT[r0:r0 + D, jt * P:(jt + 1) * P],
                    qT[r0:r0 + D, i0:i0 + CH],
                    start=True, stop=True,
                    tile_position=(r0, 0),
                )

        def emit_exp(g):
            r = g % RB
            nc.scalar.activation(
                pT[:, 2 * r:2 * r + 2, :],
                qk_ps[:, r * 2 * CH:(r + 1) * 2 * CH],
                ACTF.Exp, bias=nbias_sb, scale=SCALE,
            )

        def emit_pv(j):
            c, jt2 = divmod(j, JT2)
            if jt2 == 0:
                attn_tiles[c] = ps_attn.tile([D + 1, CH], F32, tag="attn",
                                             name=f"attn_{c}")
            m = j % RB
            nc.tensor.matmul(
                attn_tiles[c],
                v_aug[:, jt2, :, 0:D + 1],
                pT[:, 2 * m:2 * m + 2, :],
                start=(jt2 == 0), stop=(jt2 == JT2 - 1),
                perf_mode=mybir.MatmulPerfMode.DoubleRow,
            )

        def tail_a(c):
            # evacuate attn accumulator promptly: with a single-buffered attn
            # bank, PV(c+1) start waits on this read.
            attn_sb = work.tile([D + 1, CH], F32, tag="attn_sb")
            nc.vector.tensor_copy(attn_sb, attn_tiles.pop(c))
            chunk_state[c] = attn_sb

        def tail_b(c):
            attn_sb = chunk_state[c]
            recip_sb = work.tile([1, CH], F32, tag="recip")
            nc.vector.reciprocal(recip_sb, attn_sb[D:D + 1, :])
            i0 = c * CH
            recipb_ps = tail_ps[0:D, :]
            nc.tensor.matmul(recipb_ps, ones_sb, recip_sb, start=True, stop=True)
            tmp = work.tile([D, CH], BF16, tag="tmp")
            diffT = work.tile([D, CH], BF16, tag="diffT")
            with nc.allow_low_precision(reason="dif branch tolerates bf16"):
                nc.vector.tensor_mul(tmp, attn_sb[0:D, :], recipb_ps)
                nc.vector.tensor_sub(diffT, vT[:, i0:i0 + CH], tmp)
            chunk_state[c] = diffT

        def tail_c(c):
            diffT = chunk_state.pop(c)
            v1_ps = tail_ps[:, 0:TPC * D]
            for s in range(TPC):
                nc.tensor.matmul(
                    v1_ps[:, s * D:(s + 1) * D],
                    diffT[:, s * P:(s + 1) * P],
                    wdif_sb,
                    start=True, stop=True,
                )
            with nc.allow_low_precision(reason="v1 tolerates bf16"):
                nc.vector.tensor_add(v1_nat[:, c * TPC:(c + 1) * TPC, :],
                                     v1_ps, bdif_bc)
            T_part = tail_ps[0:D, TPC * D:TPC * D + D]
            for s in range(TPC):
                t = c * TPC + s
                nc.tensor.matmul(
                    T_part,
                    k_bf[:, t, :],
                    v1_nat[:, t, :],
                    start=(s == 0), stop=(s == TPC - 1),
                )
            nc.vector.tensor_add(T_sb, T_sb, T_part)

        def tr_q_group(c):
            # build qT for chunk c (tiles 4c..4c+3) through the tail bank
            nc.vector.tensor_copy(q_bf[:, c * TPC:(c + 1) * TPC, :],
                                  q_nat[:, c * TPC:(c + 1) * TPC, :])
            ptw = tail_ps[0:D, :]  # [64, 512] f32
            for s in range(TPC):
                t = c * TPC + s
                nc.tensor.matmul(ptw[:, s * P:(s + 1) * P], q_bf[:, t, :], ident_bf,
                                 start=True, stop=True)
            nc.vector.tensor_copy(qT[0:D, c * CH:(c + 1) * CH], ptw)
            nc.gpsimd.tensor_copy(qT[D:P, c * CH:(c + 1) * CH],
                                  qT[0:D, c * CH:(c + 1) * CH])

        def tr_v_group(c):
            # build vT for chunk c (tiles 4c..4c+3)
            ptw = tail_ps[0:D, :]
            for s in range(TPC):
                t = c * TPC + s
                nc.tensor.matmul(ptw[:, s * P:(s + 1) * P], v_nat[:, t, :], ident,
                                 start=True, stop=True)
            nc.vector.tensor_copy(vT[:, c * CH:(c + 1) * CH], ptw)

        # pipelined emission over 128 global iterations
        for g in range(G):
            c, jt2 = divmod(g, JT2)
            emit_qk(g)
            emit_exp(g)
            if g >= 2:
                emit_pv(g - 2)
            # per-chunk tails on the previous chunk, staggered
            if c >= 1:
                if jt2 == 3:
                    tail_a(c - 1)
                elif jt2 == 6:
                    tail_b(c - 1)
                elif jt2 == 9:
                    tail_c(c - 1)
            # deferred transposes through the tail bank
            if jt2 == 12 and c + 1 < NCH:
                tr_q_group(c + 1)
            if jt2 == 14 and c < NCH:
                tr_v_group(c)
        emit_pv(G - 2)
        emit_pv(G - 1)
        tail_a(NCH - 1)
        tail_b(NCH - 1)
        tail_c(NCH - 1)

    # T picks up the deferred 1/sqrt(D) score scaling; dual-packed bf16 copy
    # for the pass-2 v_new matmuls (tile_position halves need partition-
    # matched rhs operands).
    T_bf = consts.tile([D, D], BF16, tag="T_bf")
    with nc.allow_low_precision(reason="v_new correction tolerates bf16"):
        nc.vector.tensor_scalar_mul(T_bf, T_sb, SCALE)

    # ---------------- pass 2: v_new, LN, MLP, residual ----------------
    with ExitStack() as p2:
        ps_vn = p2.enter_context(tc.tile_pool(name="ps_vn", bufs=1, space="PSUM"))
        ps_ht = p2.enter_context(tc.tile_pool(name="ps_ht", bufs=1, space="PSUM"))
        ps_z1 = p2.enter_context(tc.tile_pool(name="ps_z1", bufs=2, space="PSUM"))
        p2w = p2.enter_context(tc.tile_pool(name="p2w", bufs=4))

        state = {}

        def s12(c):
            # v_new = scale * q @ T + q, then LN stats + normalized h
            vn_ps = ps_vn.tile([P, TPC * D], F32, tag="vn")
            for s in range(TPC):
                t = c * TPC + s
                nc.tensor.matmul(
                    vn_ps[:, s * D:(s + 1) * D],
                    qT[0:D, t * P:(t + 1) * P],
                    T_bf,
                    start=True, stop=True,
                )
            v_new = p2w.tile([P, TPC, D], F32, tag="v_new")
            nc.vector.tensor_add(v_new, vn_ps, q_nat[:, c * TPC:(c + 1) * TPC, :])

            stats = p2w.tile([P, TPC, 6], F32, tag="stats")
            mv = p2w.tile([P, TPC, 2], F32, tag="mv")
            for s in range(TPC):
                nc.vector.bn_stats(stats[:, s, :], v_new[:, s, :])
                nc.vector.bn_aggr(mv[:, s, :], stats[:, s, :])
            rstd = p2w.tile([P, TPC], F32, tag="rstd")
            nc.scalar.activation(rstd, mv[:, :, 1], ACTF.Sqrt, bias=eps_sb)
            nc.vector.reciprocal(rstd, rstd)

            h = p2w.tile([P, TPC, D], BF16, tag="h")
            for s in range(TPC):
                nc.vector.tensor_scalar(
                    h[:, s, :], v_new[:, s, :],
                    scalar1=mv[:, s, 0:1], scalar2=rstd[:, s:s + 1],
                    op0=ALU.subtract, op1=ALU.mult,
                )
            state[c] = (v_new, h)

        def s3(c):
            # h^T via regular matmuls vs identity, then the MLP up-projection
            v_new, h = state[c]
            hT_ps = ps_ht.tile([D, CH], F32, tag="hT")
            for s in range(TPC):
                nc.tensor.matmul(hT_ps[:, s * P:(s + 1) * P], h[:, s, :], ident_bf,
                                 start=True, stop=True)
            hT = p2w.tile([D, CH], BF16, tag="hTsb")
            nc.vector.tensor_copy(hT, hT_ps)
            z1_ps = ps_z1.tile([P, 2 * CH], F32, tag="z1")
            for a in range(2):
                nc.tensor.matmul(
                    z1_ps[:, a * CH:(a + 1) * CH],
                    w1p_sb[:, a * P:(a + 1) * P],
                    hT,
                    start=True, stop=True,
                )
            state[c] = (v_new, z1_ps)

        def s5(c):
            v_new, z1_ps = state.pop(c)
            g1 = p2w.tile([P, 2, CH], BF16, tag="g1")
            for a in range(2):
                nc.scalar.activation(
                    g1[:, a, :], z1_ps[:, a * CH:(a + 1) * CH],
                    ACTF.Gelu, bias=b1p_sb[:, a:a + 1],
                )
            mlp_ps = ps_vn.tile([P, TPC * D], F32, tag="mlp")
            for s in range(TPC):
                for a in range(2):
                    nc.tensor.matmul(
                        mlp_ps[:, s * D:(s + 1) * D],
                        g1[:, a, s * P:(s + 1) * P],
                        w2_sb[:, a, :],
                        start=(a == 0), stop=(a == 1),
                    )
            o1 = p2w.tile([P, TPC, D], F32, tag="o1")
            nc.vector.tensor_add(o1, mlp_ps, v_new)
            o2 = p2w.tile([P, TPC, D], F32, tag="o2")
            nc.vector.tensor_add(o2, o1, b2_bc)
            nc.sync.dma_start(
                out=out.rearrange("(t p) d -> p t d", p=P)[:, c * TPC:(c + 1) * TPC, :],
                in_=o2,
            )

        for step in range(NCH + 2):
            if step < NCH:
                s12(step)
            if 0 <= step - 1 < NCH:
                s3(step - 1)
            if 0 <= step - 2 < NCH:
                s5(step - 2)


_NC_CACHE = None


def _get_nc():
    global _NC_CACHE
    if _NC_CACHE is None:
        _NC_CACHE = build_nc()
    return _NC_CACHE


def kernel(**inputs) -> np.ndarray:
    nc = _get_nc()
    per_batch = {"q", "k", "v"}
    in_maps = []
    for b in range(B):
        m = {}
        for name, arr in inputs.items():
            arr = np.asarray(arr)
            m[name] = np.ascontiguousarray(arr[b] if name in per_batch else arr)
        in_maps.append(m)
    res = run_bass_kernel_spmd(nc, in_maps, core_ids=list(range(B)))
    return np.stack([res.results[i]["out"] for i in range(B)], axis=0)



# revision 2
# speedup vs baseline: 1.4093x; 1.4093x over previous
"""Trainium2 Bass kernel for nn_Difference_Module (dense transformer block).

Math (per batch, N=4096, D=64, H=256):
    S      = q @ k^T / 8                       [N, N]
    attn   = softmax(S) @ v                    [N, D]
    v1     = (v - attn) @ W_dif + b_dif        [N, D]
    v_new  = S @ v1 + q
    h      = layernorm(v_new) * gamma + beta
    out    = gelu(h @ W1 + b1) @ W2 + b2 + v_new

Key algebraic optimization: S is rank-64 (S = q @ k^T / 8), so
    S @ v1 = q @ (k^T @ v1) / 8
which removes any need to materialize or recompute S for the second use.
Only the softmax path touches the full [N, N] score matrix, flash-style:
we compute S^T tiles (k-index on partitions, q-index on the free axis),
exponentiate without max-subtraction (scores ~ N(0,1), no overflow), and
accumulate exp(S)^T-weighted V with an appended ones-column to get the
softmax denominators in the same matmul.

The scalar (ACT) engine is the bottleneck: 16.8M exps at ~1 elem/lane/
cycle. Everything else is arranged to hide under it:
  - exp runs in [128, 1536] units (amortizes the ~352-cycle per-
    instruction overhead) out of a manually rotated 6-bank PSUM region.
  - all transposes are regular matmuls against an identity (pipelines at
    ~85ns/tile vs ~350ns for transpose-mode), mostly bf16.
  - pass-2 rstd = exp(-0.5*ln(var+eps)) stays in the natural_log_exp
    ACT table set (pinned by a dummy ln at init), so the only table
    switch in the whole kernel is the one load for gelu.
  - softmax denominators use the fast Newton-Raphson reciprocal.

Sharding: pure data parallel, one batch per NeuronCore (B=8, 8 cores),
no collectives.
"""

import sys
from contextlib import ExitStack

import numpy as np

for _p in ("/opt/trn_rl_repo",):
    if _p not in sys.path:
        sys.path.insert(0, _p)

import concourse.bass as bass
import concourse.bacc as bacc
import concourse.tile as tile
from concourse import mybir
from concourse.bass_utils import run_bass_kernel_spmd
from concourse.masks import make_identity

N = 4096          # sequence length per batch
D = 64            # model dim
H = 256           # mlp hidden dim
B = 8             # batches == cores
P = 128           # SBUF partitions
NT = N // P       # 32 row-tiles of 128
CH = 512          # chunk of the q/free axis
NCH = N // CH     # 8 chunks
TPC = CH // P     # 4 row-tiles per chunk
JT2 = NT // 2     # 16 QK iterations per chunk (2 j-tiles each)
G = NCH * JT2     # 128 global QK iterations
RB = 3            # QK psum region holds 3 iterations (3 * 1024 f32 = 6 banks)
EXPU = 1536       # exp unit: half the region
EPS = 1e-5
SCALE = 0.125     # 1/sqrt(D)

F32 = mybir.dt.float32
F32R = mybir.dt.float32r
BF16 = mybir.dt.bfloat16
FP8 = mybir.dt.float8e4
ALU = mybir.AluOpType
ACTF = mybir.ActivationFunctionType


def build_nc() -> bass.Bass:
    nc = bacc.Bacc("TRN2", target_bir_lowering=False, debug=False, num_devices=B)

    q = nc.dram_tensor("q", [N, D], F32, kind="ExternalInput").ap()
    k = nc.dram_tensor("k", [N, D], F32, kind="ExternalInput").ap()
    v = nc.dram_tensor("v", [N, D], F32, kind="ExternalInput").ap()
    w_dif = nc.dram_tensor("W_dif", [D, D], F32, kind="ExternalInput").ap()
    b_dif = nc.dram_tensor("b_dif", [D], F32, kind="ExternalInput").ap()
    gamma = nc.dram_tensor("gamma", [D], F32, kind="ExternalInput").ap()
    beta = nc.dram_tensor("beta", [D], F32, kind="ExternalInput").ap()
    w1 = nc.dram_tensor("W1", [D, H], F32, kind="ExternalInput").ap()
    b1 = nc.dram_tensor("b1", [H], F32, kind="ExternalInput").ap()
    w2 = nc.dram_tensor("W2", [H, D], F32, kind="ExternalInput").ap()
    b2 = nc.dram_tensor("b2", [D], F32, kind="ExternalInput").ap()
    out = nc.dram_tensor("out", [N, D], F32, kind="ExternalOutput").ap()

    with tile.TileContext(nc) as tc:
        with ExitStack() as ctx:
            _body(ctx, tc, q, k, v, w_dif, b_dif, gamma, beta, w1, b1, w2, b2, out)
    nc.compile()
    return nc


def _bcast_free(nc, dst, src_dram):
    """DMA a [D] dram vector into dst [P, reps, D]: broadcast on partitions,
    replicated `reps` times along the free axis."""
    reps = dst.shape[1]
    for i in range(reps):
        nc.sync.dma_start(
            out=dst[:, i, :],
            in_=bass.AP(
                tensor=src_dram.tensor,
                offset=src_dram.offset,
                ap=[[0, P]] + src_dram.ap,
            ),
        )


def _body(ctx, tc, q, k, v, w_dif, b_dif, gamma, beta, w1, b1, w2, b2, out):
    nc = tc.nc

    consts = ctx.enter_context(tc.tile_pool(name="consts", bufs=1))
    big = ctx.enter_context(tc.tile_pool(name="big", bufs=1))
    work = ctx.enter_context(tc.tile_pool(name="work", bufs=4))

    # ---------------- DMA loads (k first: it gates everything) ----------
    k_nat = big.tile([P, NT, D], F32, tag="k_nat")
    q_nat = big.tile([P, NT, D], F32, tag="q_nat")
    v_nat = big.tile([P, NT, D], F32, tag="v_nat")
    GBD = 8
    k_rr = k.rearrange("(t p) d -> p t d", p=P)
    q_rr = q.rearrange("(t p) d -> p t d", p=P)
    v_rr = v.rearrange("(t p) d -> p t d", p=P)
    for g in range(NT // GBD):
        nc.sync.dma_start(out=k_nat[:, g * GBD:(g + 1) * GBD, :],
                          in_=k_rr[:, g * GBD:(g + 1) * GBD, :])
    nc.sync.dma_start(out=q_nat[:, 0:GBD, :], in_=q_rr[:, 0:GBD, :])
    for g in range(NT // GBD):
        nc.sync.dma_start(out=v_nat[:, g * GBD:(g + 1) * GBD, :],
                          in_=v_rr[:, g * GBD:(g + 1) * GBD, :])
    for g in range(1, NT // GBD):
        nc.sync.dma_start(out=q_nat[:, g * GBD:(g + 1) * GBD, :],
                          in_=q_rr[:, g * GBD:(g + 1) * GBD, :])

    # ---------------- constants / parameters ----------------
    ident = consts.tile([P, P], F32, tag="ident")
    make_identity(nc, ident)
    ident_bf = consts.tile([P, P], BF16, tag="ident_bf")
    nc.vector.tensor_copy(ident_bf, ident)

    wdif_f = consts.tile([D, D], F32, tag="wdif_f")
    nc.sync.dma_start(out=wdif_f, in_=w_dif)
    wdif_sb = consts.tile([D, D], BF16, tag="wdif")
    nc.vector.tensor_copy(wdif_sb, wdif_f)

    w1_sb = consts.tile([D, H], F32, tag="w1")
    nc.sync.dma_start(out=w1_sb, in_=w1)
    gamma_sb = consts.tile([D, 1], F32, tag="gamma")
    nc.sync.dma_start(out=gamma_sb, in_=gamma[:, None])
    beta_sb = consts.tile([D, 1], F32, tag="beta")
    nc.sync.dma_start(out=beta_sb, in_=beta[:, None])

    # Fold LN gamma into W1 (h_hat * gamma @ W1 = h_hat @ (gamma[:,None]*W1));
    # beta's contribution lands in the bias: b1' = b1 + beta @ W1.
    w1p_sb = consts.tile([D, H], BF16, tag="w1p")
    nc.vector.tensor_scalar_mul(w1p_sb, w1_sb, gamma_sb)

    b1_sb = consts.tile([P, 2], F32, tag="b1")
    nc.sync.dma_start(out=b1_sb, in_=b1.rearrange("(a p) -> p a", p=P))

    w2f_sb = consts.tile([P, 2, D], F32, tag="w2f")
    nc.sync.dma_start(out=w2f_sb, in_=w2.rearrange("(a p) d -> p a d", p=P))
    w2_sb = consts.tile([P, 2, D], BF16, tag="w2")
    nc.vector.tensor_copy(w2_sb, w2f_sb)

    b2_bc = consts.tile([P, TPC, D], F32, tag="b2bc")
    _bcast_free(nc, b2_bc, b2)
    bdif_bc = consts.tile([P, TPC, D], F32, tag="bdifbc")
    _bcast_free(nc, bdif_bc, b_dif)

    ones_sb = consts.tile([1, D], F32, tag="ones")
    nc.vector.memset(ones_sb, 1.0)
    eps_sb = consts.tile([P, 1], F32, tag="eps")
    nc.vector.memset(eps_sb, EPS)
    nbias_sb = consts.tile([P, 1], F32, tag="nbias")
    nc.vector.memset(nbias_sb, -2.5)


    # ---------------- bf16 copies + transposed layouts ----------------
    k_bf = big.tile([P, NT, D], BF16, tag="k_bf")
    q_bf = big.tile([P, NT, D], BF16, tag="q_bf")

    qT = big.tile([P, N], BF16, tag="qT")   # rows 0-63 and 64-127 both hold q^T
    kT = big.tile([P, N], BF16, tag="kT")   # rows 0-63 and 64-127 both hold k^T
    vT = big.tile([D, N], BF16, tag="vT")

    b1p_sb = consts.tile([P, 2], F32, tag="b1p")

    # init-phase PSUM pool: b1p matmuls + k transposes + q chunk-0 transpose.
    # Closed before the pass-1 pools open so the banks get reused.
    with ExitStack() as sctx:
        ps_init = sctx.enter_context(tc.tile_pool(name="ps_init", bufs=2, space="PSUM"))
        for a in range(2):
            bw = ps_init.tile([P, 1], F32, tag="bw")
            nc.tensor.matmul(
                bw, w1_sb[:, a * P:(a + 1) * P], beta_sb, start=True, stop=True
            )
            nc.vector.tensor_add(b1p_sb[:, a:a + 1], bw, b1_sb[:, a:a + 1])

        # k transposes: 4 groups of 8 tiles (cast -> matmul-transpose -> evac,
        # pipelined per DMA piece), regular matmuls vs identity
        for gidx in range(4):
            nc.vector.tensor_copy(k_bf[:, gidx * 8:(gidx + 1) * 8, :],
                                  k_nat[:, gidx * 8:(gidx + 1) * 8, :])
            pt = ps_init.tile([D, 8 * P], F32, tag="ktr", name=f"ktr{gidx}")
            for s in range(8):
                t = gidx * 8 + s
                nc.tensor.matmul(pt[:, s * P:(s + 1) * P], k_bf[:, t, :], ident_bf,
                                 start=True, stop=True)
            nc.vector.tensor_copy(kT[0:D, gidx * 8 * P:(gidx + 1) * 8 * P], pt)
        # duplicate k^T into rows 64..127 on the idle gpsimd engine
        nc.gpsimd.tensor_copy(kT[D:P, :], kT[0:D, :])
        nc.vector.tensor_copy(q_bf[:, 0:TPC, :], q_nat[:, 0:TPC, :])

        # q chunk 0 transpose (4 tiles)
        pt = ps_init.tile([D, TPC * P], F32, tag="qtr0")
        for s in range(TPC):
            nc.tensor.matmul(pt[:, s * P:(s + 1) * P], q_bf[:, s, :], ident_bf,
                             start=True, stop=True)
        nc.vector.tensor_copy(qT[0:D, 0:CH], pt)
        nc.gpsimd.tensor_copy(qT[D:P, 0:CH], qT[0:D, 0:CH])

    # V with an appended ones column: the PV matmul then also produces the
    # softmax denominators (row 64 of the accumulator).
    # DoubleRow fp8 layout: pairs of j-tiles interleaved on the ko axis,
    # inner stride padded to 80 bytes (16-aligned). Ones column -> denominators.
    v_aug = big.tile([P, NT // 2, 2, 80], FP8, tag="v_aug")
    with nc.allow_low_precision(reason="softmax-averaged fp8 PV"):
        for gidx in range(4):
            nc.vector.tensor_copy(v_aug[:, gidx * 4:(gidx + 1) * 4, :, 0:D],
                                  v_nat[:, gidx * 8:(gidx + 1) * 8, :])
    nc.vector.memset(v_aug[:, :, :, D:D + 1], 1.0)

    v1_nat = big.tile([P, NT, D], BF16, tag="v1_nat")
    T_sb = big.tile([D, D], F32, tag="T_sb")
    nc.vector.memset(T_sb, 0.0)
    pT = big.tile([P, 2 * RB, CH], FP8, tag="pT")   # rotating exp output

    # ---------------- pass 1: flash attention + dif_proj + T ----------------
    with ExitStack() as p1:
        # One 2-bank tile per in-flight QK region (bufs=RB rotation). A single
        # manually-sliced 6-bank tile gets whole-tile dependency tracking:
        # QK(g) then waits on exp(g-1) instead of exp(g-RB), serializing the
        # exp <-> QK pipeline.
        ps_qk_pool = p1.enter_context(tc.tile_pool(name="ps_qk", bufs=RB, space="PSUM"))
        ps_attn = p1.enter_context(tc.tile_pool(name="ps_attn", bufs=1, space="PSUM"))
        ps_tail_pool = p1.enter_context(tc.tile_pool(name="ps_tail", bufs=1, space="PSUM"))

        tail_ps = ps_tail_pool.tile([P, CH], F32, tag="tail")       # 1 bank

        qk_tiles = {}
        attn_tiles = {}
        chunk_state = {}

        def emit_qk(g):
            c, jt2 = divmod(g, JT2)
            st = ps_qk_pool.tile([P, 2 * CH], F32, tag="qk", name=f"qk{g}")
            qk_tiles[g] = st
            i0 = c * CH
            for s in range(2):
                jt = jt2 * 2 + s
                r0 = s * D
                nc.tensor.matmul(
                    st[:, s * CH:(s + 1) * CH],
                    kT[r0:r0 + D, jt * P:(jt + 1) * P],
                    qT[r0:r0 + D, i0:i0 + CH],
                    start=True, stop=True,
                    tile_position=(r0, 0),
                )

        def emit_exp(g):
            r = g % RB
            nc.scalar.activation(
                pT[:, 2 * r:2 * r + 2, :],
                qk_tiles.pop(g),
                ACTF.Exp, bias=nbias_sb, scale=SCALE,
            )

        def emit_pv(j):
            c, jt2 = divmod(j, JT2)
            if jt2 == 0:
                attn_tiles[c] = ps_attn.tile([D + 1, CH], F32, tag="attn",
                                             name=f"attn_{c}")
            m = j % RB
            nc.tensor.matmul(
                attn_tiles[c],
                v_aug[:, jt2, :, 0:D + 1],
                pT[:, 2 * m:2 * m + 2, :],
                start=(jt2 == 0), stop=(jt2 == JT2 - 1),
                perf_mode=mybir.MatmulPerfMode.DoubleRow,
            )

        def tail_a(c):
            # evacuate attn accumulator promptly: with a single-buffered attn
            # bank, PV(c+1) start waits on this read.
            attn_sb = work.tile([D + 1, CH], F32, tag="attn_sb")
            nc.vector.tensor_copy(attn_sb, attn_tiles.pop(c))
            chunk_state[c] = attn_sb

        def tail_b(c):
            attn_sb = chunk_state[c]
            recip_sb = work.tile([1, CH], F32, tag="recip")
            nc.vector.reciprocal(recip_sb, attn_sb[D:D + 1, :])
            i0 = c * CH
            recipb_ps = tail_ps[0:D, :]
            nc.tensor.matmul(recipb_ps, ones_sb, recip_sb, start=True, stop=True)
            tmp = work.tile([D, CH], BF16, tag="tmp")
            diffT = work.tile([D, CH], BF16, tag="diffT")
            with nc.allow_low_precision(reason="dif branch tolerates bf16"):
                nc.vector.tensor_mul(tmp, attn_sb[0:D, :], recipb_ps)
                nc.vector.tensor_sub(diffT, vT[:, i0:i0 + CH], tmp)
            chunk_state[c] = diffT

        def tail_c(c):
            diffT = chunk_state.pop(c)
            v1_ps = tail_ps[:, 0:TPC * D]
            for s in range(TPC):
                nc.tensor.matmul(
                    v1_ps[:, s * D:(s + 1) * D],
                    diffT[:, s * P:(s + 1) * P],
                    wdif_sb,
                    start=True, stop=True,
                )
            with nc.allow_low_precision(reason="v1 tolerates bf16"):
                nc.vector.tensor_add(v1_nat[:, c * TPC:(c + 1) * TPC, :],
                                     v1_ps, bdif_bc)
            T_part = tail_ps[0:D, TPC * D:TPC * D + D]
            for s in range(TPC):
                t = c * TPC + s
                nc.tensor.matmul(
                    T_part,
                    k_bf[:, t, :],
                    v1_nat[:, t, :],
                    start=(s == 0), stop=(s == TPC - 1),
                )
            nc.vector.tensor_add(T_sb, T_sb, T_part)

        def tr_q_group(c):
            # build qT for chunk c (tiles 4c..4c+3) through the tail bank
            nc.vector.tensor_copy(q_bf[:, c * TPC:(c + 1) * TPC, :],
                                  q_nat[:, c * TPC:(c + 1) * TPC, :])
            ptw = tail_ps[0:D, :]  # [64, 512] f32
            for s in range(TPC):
                t = c * TPC + s
                nc.tensor.matmul(ptw[:, s * P:(s + 1) * P], q_bf[:, t, :], ident_bf,
                                 start=True, stop=True)
            nc.vector.tensor_copy(qT[0:D, c * CH:(c + 1) * CH], ptw)
            nc.gpsimd.tensor_copy(qT[D:P, c * CH:(c + 1) * CH],
                                  qT[0:D, c * CH:(c + 1) * CH])

        def tr_v_group(c):
            # build vT for chunk c (tiles 4c..4c+3)
            ptw = tail_ps[0:D, :]
            for s in range(TPC):
                t = c * TPC + s
                nc.tensor.matmul(ptw[:, s * P:(s + 1) * P], v_nat[:, t, :], ident,
                                 start=True, stop=True)
            nc.vector.tensor_copy(vT[:, c * CH:(c + 1) * CH], ptw)

        # pipelined emission over 128 global iterations
        for g in range(G):
            c, jt2 = divmod(g, JT2)
            emit_qk(g)
            emit_exp(g)
            if g >= 2:
                emit_pv(g - 2)
            # per-chunk tails on the previous chunk, staggered
            if c >= 1:
                if jt2 == 3:
                    tail_a(c - 1)
                elif jt2 == 6:
                    tail_b(c - 1)
                elif jt2 == 9:
                    tail_c(c - 1)
            # deferred transposes through the tail bank
            if jt2 == 12 and c + 1 < NCH:
                tr_q_group(c + 1)
            if jt2 == 14 and c < NCH:
                tr_v_group(c)
        emit_pv(G - 2)
        emit_pv(G - 1)
        tail_a(NCH - 1)
        tail_b(NCH - 1)
        tail_c(NCH - 1)

    # T picks up the deferred 1/sqrt(D) score scaling; dual-packed bf16 copy
    # for the pass-2 v_new matmuls (tile_position halves need partition-
    # matched rhs operands).
    T_bf = consts.tile([D, D], BF16, tag="T_bf")
    with nc.allow_low_precision(reason="v_new correction tolerates bf16"):
        nc.vector.tensor_scalar_mul(T_bf, T_sb, SCALE)

    # ---------------- pass 2: v_new, LN, MLP, residual ----------------
    with ExitStack() as p2:
        ps_vn = p2.enter_context(tc.tile_pool(name="ps_vn", bufs=1, space="PSUM"))
        ps_ht = p2.enter_context(tc.tile_pool(name="ps_ht", bufs=1, space="PSUM"))
        ps_z1 = p2.enter_context(tc.tile_pool(name="ps_z1", bufs=2, space="PSUM"))
        p2w = p2.enter_context(tc.tile_pool(name="p2w", bufs=4))

        state = {}

        def s12(c):
            # v_new = scale * q @ T + q, then LN stats + normalized h
            vn_ps = ps_vn.tile([P, TPC * D], F32, tag="vn")
            for s in range(TPC):
                t = c * TPC + s
                nc.tensor.matmul(
                    vn_ps[:, s * D:(s + 1) * D],
                    qT[0:D, t * P:(t + 1) * P],
                    T_bf,
                    start=True, stop=True,
                )
            v_new = p2w.tile([P, TPC, D], F32, tag="v_new")
            nc.vector.tensor_add(v_new, vn_ps, q_nat[:, c * TPC:(c + 1) * TPC, :])

            stats = p2w.tile([P, TPC, 6], F32, tag="stats")
            mv = p2w.tile([P, TPC, 2], F32, tag="mv")
            for s in range(TPC):
                nc.vector.bn_stats(stats[:, s, :], v_new[:, s, :])
                nc.vector.bn_aggr(mv[:, s, :], stats[:, s, :])
            rstd = p2w.tile([P, TPC], F32, tag="rstd")
            nc.scalar.activation(rstd, mv[:, :, 1], ACTF.Sqrt, bias=eps_sb)
            nc.vector.reciprocal(rstd, rstd)

            h = p2w.tile([P, TPC, D], BF16, tag="h")
            for s in range(TPC):
                nc.vector.tensor_scalar(
                    h[:, s, :], v_new[:, s, :],
                    scalar1=mv[:, s, 0:1], scalar2=rstd[:, s:s + 1],
                    op0=ALU.subtract, op1=ALU.mult,
                )
            state[c] = (v_new, h)

        def s3(c):
            # h^T via regular matmuls vs identity, then the MLP up-projection
            v_new, h = state[c]
            hT_ps = ps_ht.tile([D, CH], F32, tag="hT")
            for s in range(TPC):
                nc.tensor.matmul(hT_ps[:, s * P:(s + 1) * P], h[:, s, :], ident_bf,
                                 start=True, stop=True)
            hT = p2w.tile([D, CH], BF16, tag="hTsb")
            nc.vector.tensor_copy(hT, hT_ps)
            z1_ps = ps_z1.tile([P, 2 * CH], F32, tag="z1")
            for a in range(2):
                nc.tensor.matmul(
                    z1_ps[:, a * CH:(a + 1) * CH],
                    w1p_sb[:, a * P:(a + 1) * P],
                    hT,
                    start=True, stop=True,
                )
            state[c] = (v_new, z1_ps)

        def s5(c):
            v_new, z1_ps = state.pop(c)
            g1 = p2w.tile([P, 2, CH], BF16, tag="g1")
            for a in range(2):
                nc.scalar.activation(
                    g1[:, a, :], z1_ps[:, a * CH:(a + 1) * CH],
                    ACTF.Gelu, bias=b1p_sb[:, a:a + 1],
                )
            mlp_ps = ps_vn.tile([P, TPC * D], F32, tag="mlp")
            for s in range(TPC):
                for a in range(2):
                    nc.tensor.matmul(
                        mlp_ps[:, s * D:(s + 1) * D],
                        g1[:, a, s * P:(s + 1) * P],
                        w2_sb[:, a, :],
                        start=(a == 0), stop=(a == 1),
                    )
            o1 = p2w.tile([P, TPC, D], F32, tag="o1")
            nc.vector.tensor_add(o1, mlp_ps, v_new)
            o2 = p2w.tile([P, TPC, D], F32, tag="o2")
            nc.vector.tensor_add(o2, o1, b2_bc)
            nc.sync.dma_start(
                out=out.rearrange("(t p) d -> p t d", p=P)[:, c * TPC:(c + 1) * TPC, :],
                in_=o2,
            )

        for step in range(NCH + 2):
            if step < NCH:
                s12(step)
            if 0 <= step - 1 < NCH:
                s3(step - 1)
            if 0 <= step - 2 < NCH:
                s5(step - 2)


_NC_CACHE = None


def _get_nc():
    global _NC_CACHE
    if _NC_CACHE is None:
        _NC_CACHE = build_nc()
    return _NC_CACHE


def kernel(**inputs) -> np.ndarray:
    nc = _get_nc()
    per_batch = {"q", "k", "v"}
    in_maps = []
    for b in range(B):
        m = {}
        for name, arr in inputs.items():
            arr = np.asarray(arr)
            m[name] = np.ascontiguousarray(arr[b] if name in per_batch else arr)
        in_maps.append(m)
    res = run_bass_kernel_spmd(nc, in_maps, core_ids=list(range(B)))
    return np.stack([res.results[i]["out"] for i in range(B)], axis=0)



# revision 9
# speedup vs baseline: 1.4192x; 1.0070x over previous
"""Trainium2 Bass kernel for nn_Difference_Module (dense transformer block).

Math (per batch, N=4096, D=64, H=256):
    S      = q @ k^T / 8                       [N, N]
    attn   = softmax(S) @ v                    [N, D]
    v1     = (v - attn) @ W_dif + b_dif        [N, D]
    v_new  = S @ v1 + q
    h      = layernorm(v_new) * gamma + beta
    out    = gelu(h @ W1 + b1) @ W2 + b2 + v_new

Key algebraic optimization: S is rank-64 (S = q @ k^T / 8), so
    S @ v1 = q @ (k^T @ v1) / 8
which removes any need to materialize or recompute S for the second use.
Only the softmax path touches the full [N, N] score matrix, flash-style:
we compute S^T tiles (k-index on partitions, q-index on the free axis),
exponentiate without max-subtraction (scores ~ N(0,1), no overflow), and
accumulate exp(S)^T-weighted V with an appended ones-column to get the
softmax denominators in the same matmul.

The scalar (ACT) engine is the bottleneck: 16.8M exps at ~1 elem/lane/
cycle. Everything else is arranged to hide under it:
  - exp runs in [128, 1536] units (amortizes the ~352-cycle per-
    instruction overhead) out of a manually rotated 6-bank PSUM region.
  - all transposes are regular matmuls against an identity (pipelines at
    ~85ns/tile vs ~350ns for transpose-mode), mostly bf16.
  - pass-2 rstd = exp(-0.5*ln(var+eps)) stays in the natural_log_exp
    ACT table set (pinned by a dummy ln at init), so the only table
    switch in the whole kernel is the one load for gelu.
  - softmax denominators use the fast Newton-Raphson reciprocal.

Sharding: pure data parallel, one batch per NeuronCore (B=8, 8 cores),
no collectives.
"""

import sys
from contextlib import ExitStack

import numpy as np

for _p in ("/opt/trn_rl_repo",):
    if _p not in sys.path:
        sys.path.insert(0, _p)

import concourse.bass as bass
import concourse.bacc as bacc
import concourse.tile as tile
from concourse import mybir
from concourse.bass_utils import run_bass_kernel_spmd
from concourse.masks import make_identity

N = 4096          # sequence length per batch
D = 64            # model dim
H = 256           # mlp hidden dim
B = 8             # batches == cores
P = 128           # SBUF partitions
NT = N // P       # 32 row-tiles of 128
CH = 512          # chunk of the q/free axis
NCH = N // CH     # 8 chunks
TPC = CH // P     # 4 row-tiles per chunk
JT2 = NT // 2     # 16 QK iterations per chunk (2 j-tiles each)
G = NCH * JT2     # 128 global QK iterations
RB = 3            # QK psum region holds 3 iterations (3 * 1024 f32 = 6 banks)
EXPU = 1536       # exp unit: half the region
EPS = 1e-5
SCALE = 0.125     # 1/sqrt(D)

F32 = mybir.dt.float32
F32R = mybir.dt.float32r
BF16 = mybir.dt.bfloat16
FP8 = mybir.dt.float8e4
ALU = mybir.AluOpType
ACTF = mybir.ActivationFunctionType


def build_nc() -> bass.Bass:
    nc = bacc.Bacc("TRN2", target_bir_lowering=False, debug=False, num_devices=B)

    q = nc.dram_tensor("q", [N, D], F32, kind="ExternalInput").ap()
    k = nc.dram_tensor("k", [N, D], F32, kind="ExternalInput").ap()
    v = nc.dram_tensor("v", [N, D], F32, kind="ExternalInput").ap()
    w_dif = nc.dram_tensor("W_dif", [D, D], F32, kind="ExternalInput").ap()
    b_dif = nc.dram_tensor("b_dif", [D], F32, kind="ExternalInput").ap()
    gamma = nc.dram_tensor("gamma", [D], F32, kind="ExternalInput").ap()
    beta = nc.dram_tensor("beta", [D], F32, kind="ExternalInput").ap()
    w1 = nc.dram_tensor("W1", [D, H], F32, kind="ExternalInput").ap()
    b1 = nc.dram_tensor("b1", [H], F32, kind="ExternalInput").ap()
    w2 = nc.dram_tensor("W2", [H, D], F32, kind="ExternalInput").ap()
    b2 = nc.dram_tensor("b2", [D], F32, kind="ExternalInput").ap()
    out = nc.dram_tensor("out", [N, D], F32, kind="ExternalOutput").ap()

    with tile.TileContext(nc) as tc:
        with ExitStack() as ctx:
            _body(ctx, tc, q, k, v, w_dif, b_dif, gamma, beta, w1, b1, w2, b2, out)
    nc.compile()
    return nc


def _bcast_free(nc, dst, src_dram):
    """DMA a [D] dram vector into dst [P, reps, D]: broadcast on partitions,
    replicated `reps` times along the free axis (one 0-stride DMA)."""
    reps = dst.shape[1]
    nc.gpsimd.dma_start(
        out=dst,
        in_=bass.AP(
            tensor=src_dram.tensor,
            offset=src_dram.offset,
            ap=[[0, P], [0, reps]] + src_dram.ap,
        ),
    )


def _body(ctx, tc, q, k, v, w_dif, b_dif, gamma, beta, w1, b1, w2, b2, out):
    nc = tc.nc

    consts = ctx.enter_context(tc.tile_pool(name="consts", bufs=1))
    big = ctx.enter_context(tc.tile_pool(name="big", bufs=1))
    work = ctx.enter_context(tc.tile_pool(name="work", bufs=4))

    # ---------------- DMA loads (k first: it gates everything) ----------
    # Few, large DMA instructions: each dma_start costs ~0.6-1.4us of issue
    # time on its queue. q/v issue from the (otherwise idle) gpsimd queue so
    # they don't serialize behind k on the sync queue.
    k_nat = big.tile([P, NT, D], F32, tag="k_nat")
    q_nat = big.tile([P, NT, D], F32, tag="q_nat")
    v_nat = big.tile([P, NT, D], F32, tag="v_nat")
    k_rr = k.rearrange("(t p) d -> p t d", p=P)
    q_rr = q.rearrange("(t p) d -> p t d", p=P)
    v_rr = v.rearrange("(t p) d -> p t d", p=P)
    for g in range(4):
        nc.sync.dma_start(out=k_nat[:, g * 8:(g + 1) * 8, :],
                          in_=k_rr[:, g * 8:(g + 1) * 8, :])
    nc.sync.dma_start(out=q_nat[:, 0:TPC, :], in_=q_rr[:, 0:TPC, :])
    nc.gpsimd.dma_start(out=v_nat[:, 0:16, :], in_=v_rr[:, 0:16, :])
    nc.gpsimd.dma_start(out=v_nat[:, 16:32, :], in_=v_rr[:, 16:32, :])
    nc.gpsimd.dma_start(out=q_nat[:, TPC:16, :], in_=q_rr[:, TPC:16, :])
    nc.gpsimd.dma_start(out=q_nat[:, 16:32, :], in_=q_rr[:, 16:32, :])

    # ---------------- constants / parameters ----------------
    eps_sb = consts.tile([P, 1], F32, tag="eps")
    nc.vector.memset(eps_sb, EPS)
    nbias_sb = consts.tile([P, 1], F32, tag="nbias")
    nc.vector.memset(nbias_sb, -2.5)

    # Pin the exp ACT table set immediately: the (walrus-inserted) ~1.3us
    # table load runs during the input DMAs instead of gating the first exp.
    pin_sb = consts.tile([1, 1], F32, tag="pin")
    nc.scalar.activation(pin_sb, eps_sb[0:1, :], ACTF.Exp)

    ident = consts.tile([P, P], F32, tag="ident")
    make_identity(nc, ident)
    ident_bf = consts.tile([P, P], BF16, tag="ident_bf")
    nc.vector.tensor_copy(ident_bf, ident)

    wdif_f = consts.tile([D, D], F32, tag="wdif_f")
    nc.sync.dma_start(out=wdif_f, in_=w_dif)
    wdif_sb = consts.tile([D, D], BF16, tag="wdif")
    nc.vector.tensor_copy(wdif_sb, wdif_f)

    w1_sb = consts.tile([D, H], F32, tag="w1")
    nc.sync.dma_start(out=w1_sb, in_=w1)
    gamma_sb = consts.tile([D, 1], F32, tag="gamma")
    nc.sync.dma_start(out=gamma_sb, in_=gamma[:, None])
    beta_sb = consts.tile([D, 1], F32, tag="beta")
    nc.sync.dma_start(out=beta_sb, in_=beta[:, None])

    # Fold LN gamma into W1 (h_hat * gamma @ W1 = h_hat @ (gamma[:,None]*W1));
    # beta's contribution lands in the bias: b1' = b1 + beta @ W1.
    w1p_sb = consts.tile([D, H], BF16, tag="w1p")
    nc.vector.tensor_scalar_mul(w1p_sb, w1_sb, gamma_sb)

    b1_sb = consts.tile([P, 2], F32, tag="b1")
    nc.sync.dma_start(out=b1_sb, in_=b1.rearrange("(a p) -> p a", p=P))

    w2f_sb = consts.tile([P, 2, D], F32, tag="w2f")
    nc.sync.dma_start(out=w2f_sb, in_=w2.rearrange("(a p) d -> p a d", p=P))
    w2_sb = consts.tile([P, 2, D], BF16, tag="w2")
    nc.vector.tensor_copy(w2_sb, w2f_sb)

    b2_bc = consts.tile([P, TPC, D], F32, tag="b2bc")
    _bcast_free(nc, b2_bc, b2)
    bdif_bc = consts.tile([P, TPC, D], F32, tag="bdifbc")
    _bcast_free(nc, bdif_bc, b_dif)

    ones_bf = consts.tile([1, D], BF16, tag="ones")
    nc.vector.memset(ones_bf, 1.0)


    # ---------------- bf16 copies + transposed layouts ----------------
    k_bf = big.tile([P, NT, D], BF16, tag="k_bf")
    q_bf = big.tile([P, NT, D], BF16, tag="q_bf")
    v_bf = big.tile([P, NT, D], BF16, tag="v_bf")

    qT = big.tile([P, N], BF16, tag="qT")   # rows 0-63 and 64-127 both hold q^T
    kT = big.tile([P, N], BF16, tag="kT")   # rows 0-63 and 64-127 both hold k^T
    vT = big.tile([D, N], BF16, tag="vT")

    b1p_sb = consts.tile([P, 2], F32, tag="b1p")

    # init-phase PSUM pool: b1p matmuls + k transposes + q chunk-0 transpose.
    # Closed before the pass-1 pools open so the banks get reused.
    with ExitStack() as sctx:
        ps_init = sctx.enter_context(tc.tile_pool(name="ps_init", bufs=2, space="PSUM"))
        for a in range(2):
            bw = ps_init.tile([P, 1], F32, tag="bw")
            nc.tensor.matmul(
                bw, w1_sb[:, a * P:(a + 1) * P], beta_sb, start=True, stop=True
            )
            nc.vector.tensor_add(b1p_sb[:, a:a + 1], bw, b1_sb[:, a:a + 1])

        # k transposes: 4 groups of 8 tiles (cast -> matmul-transpose -> evac,
        # pipelined per DMA piece), regular matmuls vs identity
        for gidx in range(4):
            nc.vector.tensor_copy(k_bf[:, gidx * 8:(gidx + 1) * 8, :],
                                  k_nat[:, gidx * 8:(gidx + 1) * 8, :])
            pt = ps_init.tile([D, 8 * P], F32, tag="ktr", name=f"ktr{gidx}")
            for s in range(8):
                t = gidx * 8 + s
                nc.tensor.matmul(pt[:, s * P:(s + 1) * P], k_bf[:, t, :], ident_bf,
                                 start=True, stop=True)
            nc.vector.tensor_copy(kT[0:D, gidx * 8 * P:(gidx + 1) * 8 * P], pt)
            # duplicate k^T into rows 64..127 (cheap bf16 DVE copy per group)
            nc.vector.tensor_copy(kT[D:P, gidx * 8 * P:(gidx + 1) * 8 * P],
                                  kT[0:D, gidx * 8 * P:(gidx + 1) * 8 * P])
        nc.vector.tensor_copy(q_bf[:, 0:TPC, :], q_nat[:, 0:TPC, :])

        # q chunk 0 transpose (4 tiles)
        pt = ps_init.tile([D, TPC * P], F32, tag="qtr0")
        for s in range(TPC):
            nc.tensor.matmul(pt[:, s * P:(s + 1) * P], q_bf[:, s, :], ident_bf,
                             start=True, stop=True)
        nc.vector.tensor_copy(qT[0:D, 0:CH], pt)
        nc.vector.tensor_copy(qT[D:P, 0:CH], qT[0:D, 0:CH])

    # V with an appended ones column: the PV matmul then also produces the
    # softmax denominators (row 64 of the accumulator).
    # DoubleRow fp8 layout: pairs of j-tiles interleaved on the ko axis,
    # inner stride padded to 80 bytes (16-aligned). Ones column -> denominators.
    v_aug = big.tile([P, NT // 2, 2, 80], FP8, tag="v_aug")
    with nc.allow_low_precision(reason="softmax-averaged fp8 PV"):
        for gidx in range(4):
            nc.vector.tensor_copy(v_aug[:, gidx * 4:(gidx + 1) * 4, :, 0:D],
                                  v_nat[:, gidx * 8:(gidx + 1) * 8, :])
    nc.vector.memset(v_aug[:, :, :, D:D + 1], 1.0)

    v1_nat = big.tile([P, NT, D], BF16, tag="v1_nat")
    T_sb = big.tile([D, D], F32, tag="T_sb")
    nc.vector.memset(T_sb, 0.0)
    pT = big.tile([P, 2 * RB, CH], FP8, tag="pT")   # rotating exp output

    # ---------------- pass 1: flash attention + dif_proj + T ----------------
    with ExitStack() as p1:
        # One 2-bank tile per in-flight QK region (bufs=RB rotation). A single
        # manually-sliced 6-bank tile gets whole-tile dependency tracking:
        # QK(g) then waits on exp(g-1) instead of exp(g-RB), serializing the
        # exp <-> QK pipeline.
        ps_qk_pool = p1.enter_context(tc.tile_pool(name="ps_qk", bufs=RB, space="PSUM"))
        ps_attn = p1.enter_context(tc.tile_pool(name="ps_attn", bufs=1, space="PSUM"))
        ps_tail_pool = p1.enter_context(tc.tile_pool(name="ps_tail", bufs=1, space="PSUM"))

        tail_ps = ps_tail_pool.tile([P, CH], F32, tag="tail")       # 1 bank

        qk_tiles = {}
        attn_tiles = {}
        chunk_state = {}

        def emit_qk(g):
            c, jt2 = divmod(g, JT2)
            st = ps_qk_pool.tile([P, 2 * CH], F32, tag="qk", name=f"qk{g}")
            qk_tiles[g] = st
            i0 = c * CH
            for s in range(2):
                jt = jt2 * 2 + s
                r0 = s * D
                nc.tensor.matmul(
                    st[:, s * CH:(s + 1) * CH],
                    kT[r0:r0 + D, jt * P:(jt + 1) * P],
                    qT[r0:r0 + D, i0:i0 + CH],
                    start=True, stop=True,
                    tile_position=(r0, 0),
                )

        def emit_exp(g):
            r = g % RB
            nc.scalar.activation(
                pT[:, 2 * r:2 * r + 2, :],
                qk_tiles.pop(g),
                ACTF.Exp, bias=nbias_sb, scale=SCALE,
            )

        def emit_pv(j):
            c, jt2 = divmod(j, JT2)
            if jt2 == 0:
                attn_tiles[c] = ps_attn.tile([D + 1, CH], F32, tag="attn",
                                             name=f"attn_{c}")
            m = j % RB
            nc.tensor.matmul(
                attn_tiles[c],
                v_aug[:, jt2, :, 0:D + 1],
                pT[:, 2 * m:2 * m + 2, :],
                start=(jt2 == 0), stop=(jt2 == JT2 - 1),
                perf_mode=mybir.MatmulPerfMode.DoubleRow,
            )

        def tail_a(c):
            # evacuate attn accumulator promptly: with a single-buffered attn
            # bank, PV(c+1) start waits on this read.
            attn_sb = work.tile([D + 1, CH], F32, tag="attn_sb")
            nc.vector.tensor_copy(attn_sb, attn_tiles.pop(c))
            chunk_state[c] = attn_sb

        def tail_b(c):
            attn_sb = chunk_state[c]
            recip_sb = work.tile([1, CH], F32, tag="recip")
            nc.vector.reciprocal(recip_sb, attn_sb[D:D + 1, :])
            i0 = c * CH
            recipb_ps = tail_ps[0:D, :]
            # bf16 broadcast matmul: an f32 one runs as a slow LOW/HIGH 2-pass
            recip_bf = work.tile([1, CH], BF16, tag="recipbf")
            with nc.allow_low_precision(reason="softmax denom tolerates bf16"):
                nc.vector.tensor_copy(recip_bf, recip_sb)
            nc.tensor.matmul(recipb_ps, ones_bf, recip_bf, start=True, stop=True)
            tmp = work.tile([D, CH], BF16, tag="tmp")
            diffT = work.tile([D, CH], BF16, tag="diffT")
            with nc.allow_low_precision(reason="dif branch tolerates bf16"):
                nc.vector.tensor_mul(tmp, attn_sb[0:D, :], recipb_ps)
                nc.vector.tensor_sub(diffT, vT[:, i0:i0 + CH], tmp)
            chunk_state[c] = diffT

        def tail_c(c):
            diffT = chunk_state.pop(c)
            v1_ps = tail_ps[:, 0:TPC * D]
            for s in range(TPC):
                nc.tensor.matmul(
                    v1_ps[:, s * D:(s + 1) * D],
                    diffT[:, s * P:(s + 1) * P],
                    wdif_sb,
                    start=True, stop=True,
                )
            with nc.allow_low_precision(reason="v1 tolerates bf16"):
                nc.vector.tensor_add(v1_nat[:, c * TPC:(c + 1) * TPC, :],
                                     v1_ps, bdif_bc)
            T_part = tail_ps[0:D, TPC * D:TPC * D + D]
            for s in range(TPC):
                t = c * TPC + s
                nc.tensor.matmul(
                    T_part,
                    k_bf[:, t, :],
                    v1_nat[:, t, :],
                    start=(s == 0), stop=(s == TPC - 1),
                )
            nc.vector.tensor_add(T_sb, T_sb, T_part)

        def tr_q_group(c):
            # build qT for chunk c (tiles 4c..4c+3) through the tail bank
            nc.vector.tensor_copy(q_bf[:, c * TPC:(c + 1) * TPC, :],
                                  q_nat[:, c * TPC:(c + 1) * TPC, :])
            ptw = tail_ps[0:D, :]  # [64, 512] f32
            for s in range(TPC):
                t = c * TPC + s
                nc.tensor.matmul(ptw[:, s * P:(s + 1) * P], q_bf[:, t, :], ident_bf,
                                 start=True, stop=True)
            nc.vector.tensor_copy(qT[0:D, c * CH:(c + 1) * CH], ptw)
            nc.vector.tensor_copy(qT[D:P, c * CH:(c + 1) * CH],
                                  qT[0:D, c * CH:(c + 1) * CH])

        def tr_v_group(c):
            # build vT for chunk c (tiles 4c..4c+3); bf16 matmuls (f32 would
            # run as slow LOW/HIGH 2-pass)
            with nc.allow_low_precision(reason="dif branch tolerates bf16"):
                nc.vector.tensor_copy(v_bf[:, c * TPC:(c + 1) * TPC, :],
                                      v_nat[:, c * TPC:(c + 1) * TPC, :])
            ptw = tail_ps[0:D, :]
            for s in range(TPC):
                t = c * TPC + s
                nc.tensor.matmul(ptw[:, s * P:(s + 1) * P], v_bf[:, t, :], ident_bf,
                                 start=True, stop=True)
            nc.vector.tensor_copy(vT[:, c * CH:(c + 1) * CH], ptw)

        # pipelined emission over 128 global iterations
        for g in range(G):
            c, jt2 = divmod(g, JT2)
            emit_qk(g)
            emit_exp(g)
            if g >= 2:
                emit_pv(g - 2)
            # per-chunk tails on the previous chunk, staggered
            if c >= 1:
                if jt2 == 3:
                    tail_a(c - 1)
                elif jt2 == 6:
                    tail_b(c - 1)
                elif jt2 == 9:
                    tail_c(c - 1)
            # deferred transposes through the tail bank
            if jt2 == 12 and c + 1 < NCH:
                tr_q_group(c + 1)
            if jt2 == 14 and c < NCH:
                tr_v_group(c)
        emit_pv(G - 2)
        emit_pv(G - 1)
        tail_a(NCH - 1)
        tail_b(NCH - 1)
        tail_c(NCH - 1)

    # T picks up the deferred 1/sqrt(D) score scaling; dual-packed bf16 copy
    # for the pass-2 v_new matmuls (tile_position halves need partition-
    # matched rhs operands).
    T_bf = consts.tile([D, D], BF16, tag="T_bf")
    with nc.allow_low_precision(reason="v_new correction tolerates bf16"):
        nc.vector.tensor_scalar_mul(T_bf, T_sb, SCALE)

    # ---------------- pass 2: v_new, LN, MLP, residual ----------------
    # Phase A: v_new + LN stats for ALL chunks, then a single batched
    # sqrt+reciprocal (one ACT table set load instead of one per chunk),
    # then the LN applies (Identity: present in every table set).
    # Phase B: hT / z1 / gelu / mlp, gelu set loaded exactly once.
    vn_all = big.tile([P, NT, D], F32, tag="vn_all")
    h_all = big.tile([P, NT, D], BF16, tag="h_all")
    sum_t = big.tile([P, NT], F32, tag="sum_t")
    sq_t = big.tile([P, NT], F32, tag="sq_t")
    mu_t = big.tile([P, NT], F32, tag="mu_t")       # holds -mean
    var_t = big.tile([P, NT], F32, tag="var_t")
    rstd_t = big.tile([P, NT], F32, tag="rstd_t")
    nmr_t = big.tile([P, NT], F32, tag="nmr_t")     # -mean * rstd

    with ExitStack() as pa:
        ps_a = pa.enter_context(tc.tile_pool(name="ps_a", bufs=2, space="PSUM"))
        paw = pa.enter_context(tc.tile_pool(name="paw", bufs=3))
        for c in range(NCH):
            vn_ps = ps_a.tile([P, TPC * D], F32, tag="vn")
            for s in range(TPC):
                t = c * TPC + s
                nc.tensor.matmul(
                    vn_ps[:, s * D:(s + 1) * D],
                    qT[0:D, t * P:(t + 1) * P],
                    T_bf,
                    start=True, stop=True,
                )
            sl = slice(c * TPC, (c + 1) * TPC)
            nc.vector.tensor_add(vn_all[:, sl, :], vn_ps, q_nat[:, sl, :])
            nc.vector.tensor_reduce(sum_t[:, sl], vn_all[:, sl, :],
                                    axis=mybir.AxisListType.X, op=ALU.add)
            sq = paw.tile([P, TPC, D], F32, tag="sq")
            nc.vector.tensor_mul(sq, vn_all[:, sl, :], vn_all[:, sl, :])
            nc.vector.tensor_reduce(sq_t[:, sl], sq,
                                    axis=mybir.AxisListType.X, op=ALU.add)

        nc.vector.tensor_scalar_mul(mu_t, sum_t, -1.0 / D)
        nc.vector.tensor_scalar_mul(var_t, sq_t, 1.0 / D)
        musq = paw.tile([P, NT], F32, tag="musq")
        nc.vector.tensor_mul(musq, mu_t, mu_t)
        nc.vector.tensor_sub(var_t, var_t, musq)
        nc.scalar.activation(rstd_t, var_t, ACTF.Sqrt, bias=eps_sb)
        nc.vector.reciprocal(rstd_t, rstd_t)
        nc.vector.tensor_mul(nmr_t, mu_t, rstd_t)
        # LN apply h = v_new*rstd - mu*rstd, split over ACT and DVE
        for t in range(NT):
            if t % 2 == 0:
                nc.scalar.activation(
                    h_all[:, t, :], vn_all[:, t, :], ACTF.Identity,
                    bias=nmr_t[:, t:t + 1], scale=rstd_t[:, t:t + 1],
                )
            else:
                nc.vector.tensor_scalar(
                    h_all[:, t, :], vn_all[:, t, :],
                    scalar1=nmr_t[:, t:t + 1], scalar2=rstd_t[:, t:t + 1],
                    op0=ALU.add, op1=ALU.mult,
                )

    with ExitStack() as p2:
        ps_ht = p2.enter_context(tc.tile_pool(name="ps_ht", bufs=2, space="PSUM"))
        ps_z1 = p2.enter_context(tc.tile_pool(name="ps_z1", bufs=2, space="PSUM"))
        ps_mlp = p2.enter_context(tc.tile_pool(name="ps_mlp", bufs=2, space="PSUM"))
        p2w = p2.enter_context(tc.tile_pool(name="p2w", bufs=4))

        state = {}

        def s3(c):
            # h^T via regular matmuls vs identity, then the MLP up-projection
            hT_ps = ps_ht.tile([D, CH], F32, tag="hT")
            for s in range(TPC):
                t = c * TPC + s
                nc.tensor.matmul(hT_ps[:, s * P:(s + 1) * P], h_all[:, t, :],
                                 ident_bf, start=True, stop=True)
            hT = p2w.tile([D, CH], BF16, tag="hTsb")
            nc.vector.tensor_copy(hT, hT_ps)
            z1_ps = ps_z1.tile([P, 2 * CH], F32, tag="z1")
            for a in range(2):
                nc.tensor.matmul(
                    z1_ps[:, a * CH:(a + 1) * CH],
                    w1p_sb[:, a * P:(a + 1) * P],
                    hT,
                    start=True, stop=True,
                )
            state[c] = z1_ps

        def s5(c):
            z1_ps = state.pop(c)
            g1 = p2w.tile([P, 2, CH], BF16, tag="g1")
            for a in range(2):
                nc.scalar.activation(
                    g1[:, a, :], z1_ps[:, a * CH:(a + 1) * CH],
                    ACTF.Gelu, bias=b1p_sb[:, a:a + 1],
                )
            mlp_ps = ps_mlp.tile([P, TPC * D], F32, tag="mlp")
            for s in range(TPC):
                for a in range(2):
                    nc.tensor.matmul(
                        mlp_ps[:, s * D:(s + 1) * D],
                        g1[:, a, s * P:(s + 1) * P],
                        w2_sb[:, a, :],
                        start=(a == 0), stop=(a == 1),
                    )
            sl = slice(c * TPC, (c + 1) * TPC)
            o1 = p2w.tile([P, TPC, D], F32, tag="o1")
            nc.vector.tensor_add(o1, mlp_ps, vn_all[:, sl, :])
            o2 = p2w.tile([P, TPC, D], F32, tag="o2")
            nc.vector.tensor_add(o2, o1, b2_bc)
            nc.sync.dma_start(
                out=out.rearrange("(t p) d -> p t d", p=P)[:, sl, :],
                in_=o2,
            )

        for step in range(NCH + 1):
            if step < NCH:
                s3(step)
            if 0 <= step - 1 < NCH:
                s5(step - 1)


_NC_CACHE = None


def _get_nc():
    global _NC_CACHE
    if _NC_CACHE is None:
        _NC_CACHE = build_nc()
    return _NC_CACHE


def kernel(**inputs) -> np.ndarray:
    nc = _get_nc()
    per_batch = {"q", "k", "v"}
    in_maps = []
    for b in range(B):
        m = {}
        for name, arr in inputs.items():
            arr = np.asarray(arr)
            m[name] = np.ascontiguousarray(arr[b] if name in per_batch else arr)
        in_maps.append(m)
    res = run_bass_kernel_spmd(nc, in_maps, core_ids=list(range(B)))
    return np.stack([res.results[i]["out"] for i in range(B)], axis=0)



# revision 15
# speedup vs baseline: 1.5306x; 1.0785x over previous
"""Trainium2 Bass kernel for nn_Difference_Module (dense transformer block).

Math (per batch, N=4096, D=64, H=256):
    S      = q @ k^T / 8                       [N, N]
    attn   = softmax(S) @ v                    [N, D]
    v1     = (v - attn) @ W_dif + b_dif        [N, D]
    v_new  = S @ v1 + q
    h      = layernorm(v_new) * gamma + beta
    out    = gelu(h @ W1 + b1) @ W2 + b2 + v_new

Key algebraic optimization: S is rank-64 (S = q @ k^T / 8), so
    S @ v1 = q @ (k^T @ v1) / 8
which removes any need to materialize or recompute S for the second use.
Only the softmax path touches the full [N, N] score matrix, flash-style:
we compute S^T tiles (k-index on partitions, q-index on the free axis),
exponentiate without max-subtraction (scores ~ N(0,1), no overflow), and
accumulate exp(S)^T-weighted V with an appended ones-column to get the
softmax denominators in the same matmul.

The scalar (ACT) engine is the bottleneck: 16.8M exps at ~1 elem/lane/
cycle. Everything else is arranged to hide under it:
  - exp runs in [128, 1536] units (amortizes the ~352-cycle per-
    instruction overhead) out of a manually rotated 6-bank PSUM region.
  - all transposes are regular matmuls against an identity (pipelines at
    ~85ns/tile vs ~350ns for transpose-mode), mostly bf16.
  - pass-2 rstd = exp(-0.5*ln(var+eps)) stays in the natural_log_exp
    ACT table set (pinned by a dummy ln at init), so the only table
    switch in the whole kernel is the one load for gelu.
  - softmax denominators use the fast Newton-Raphson reciprocal.

Sharding: pure data parallel, one batch per NeuronCore (B=8, 8 cores),
no collectives.
"""

import sys
from contextlib import ExitStack

import numpy as np

for _p in ("/opt/trn_rl_repo",):
    if _p not in sys.path:
        sys.path.insert(0, _p)

import concourse.bass as bass
import concourse.bacc as bacc
import concourse.tile as tile
from concourse import mybir
from concourse.bass_utils import run_bass_kernel_spmd
from concourse.masks import make_identity

N = 4096          # sequence length per batch
D = 64            # model dim
H = 256           # mlp hidden dim
B = 8             # batches == cores
P = 128           # SBUF partitions
NT = N // P       # 32 row-tiles of 128
CH = 512          # chunk of the q/free axis
NCH = N // CH     # 8 chunks
TPC = CH // P     # 4 row-tiles per chunk
JT2 = NT // 2     # 16 QK iterations per chunk (2 j-tiles each)
G = NCH * JT2     # 128 global QK iterations
RB = 3            # QK psum region holds 3 iterations (3 * 1024 f32 = 6 banks)
EXPU = 1536       # exp unit: half the region
EPS = 1e-5
SCALE = 0.125     # 1/sqrt(D)

F32 = mybir.dt.float32
F32R = mybir.dt.float32r
BF16 = mybir.dt.bfloat16
FP8 = mybir.dt.float8e4
ALU = mybir.AluOpType
ACTF = mybir.ActivationFunctionType


def build_nc() -> bass.Bass:
    nc = bacc.Bacc("TRN2", target_bir_lowering=False, debug=False, num_devices=B)

    q = nc.dram_tensor("q", [N, D], F32, kind="ExternalInput").ap()
    k = nc.dram_tensor("k", [N, D], F32, kind="ExternalInput").ap()
    v = nc.dram_tensor("v", [N, D], F32, kind="ExternalInput").ap()
    w_dif = nc.dram_tensor("W_dif", [D, D], F32, kind="ExternalInput").ap()
    b_dif = nc.dram_tensor("b_dif", [D], F32, kind="ExternalInput").ap()
    gamma = nc.dram_tensor("gamma", [D], F32, kind="ExternalInput").ap()
    beta = nc.dram_tensor("beta", [D], F32, kind="ExternalInput").ap()
    w1 = nc.dram_tensor("W1", [D, H], F32, kind="ExternalInput").ap()
    b1 = nc.dram_tensor("b1", [H], F32, kind="ExternalInput").ap()
    w2 = nc.dram_tensor("W2", [H, D], F32, kind="ExternalInput").ap()
    b2 = nc.dram_tensor("b2", [D], F32, kind="ExternalInput").ap()
    out = nc.dram_tensor("out", [N, D], F32, kind="ExternalOutput").ap()

    with tile.TileContext(nc) as tc:
        with ExitStack() as ctx:
            _body(ctx, tc, q, k, v, w_dif, b_dif, gamma, beta, w1, b1, w2, b2, out)
    nc.compile()
    return nc


def _bcast_free(nc, dst, src_dram):
    """DMA a [D] dram vector into dst [P, reps, D]: broadcast on partitions,
    replicated `reps` times along the free axis (one 0-stride DMA)."""
    reps = dst.shape[1]
    nc.gpsimd.dma_start(
        out=dst,
        in_=bass.AP(
            tensor=src_dram.tensor,
            offset=src_dram.offset,
            ap=[[0, P], [0, reps]] + src_dram.ap,
        ),
    )


def _body(ctx, tc, q, k, v, w_dif, b_dif, gamma, beta, w1, b1, w2, b2, out):
    nc = tc.nc

    consts = ctx.enter_context(tc.tile_pool(name="consts", bufs=1))
    big = ctx.enter_context(tc.tile_pool(name="big", bufs=1))
    work = ctx.enter_context(tc.tile_pool(name="work", bufs=4))

    # ---------------- DMA loads (k first: it gates everything) ----------
    # Few, large DMA instructions: each dma_start costs ~0.6-1.4us of issue
    # time on its queue. q/v issue from the (otherwise idle) gpsimd queue so
    # they don't serialize behind k on the sync queue.
    # Token permutation: row (p t) of DRAM maps to partition p, tile t. This
    # makes every DMA read 1-8KB contiguous per partition (vs 256B strided) --
    # attention is invariant to a consistent permutation of q/k/v rows, and
    # the output DMA applies the inverse permutation.
    k_nat = big.tile([P, NT, D], F32, tag="k_nat")
    q_nat = big.tile([P, NT, D], F32, tag="q_nat")
    v_nat = big.tile([P, NT, D], F32, tag="v_nat")
    k_rr = k.rearrange("(p t) d -> p t d", t=NT)
    q_rr = q.rearrange("(p t) d -> p t d", t=NT)
    v_rr = v.rearrange("(p t) d -> p t d", t=NT)
    for g in range(4):
        nc.sync.dma_start(out=k_nat[:, g * 8:(g + 1) * 8, :],
                          in_=k_rr[:, g * 8:(g + 1) * 8, :])
    nc.sync.dma_start(out=q_nat[:, 0:TPC, :], in_=q_rr[:, 0:TPC, :])
    nc.sync.dma_start(out=v_nat[:, 0:16, :], in_=v_rr[:, 0:16, :])
    nc.sync.dma_start(out=v_nat[:, 16:32, :], in_=v_rr[:, 16:32, :])
    nc.sync.dma_start(out=q_nat[:, TPC:16, :], in_=q_rr[:, TPC:16, :])
    nc.sync.dma_start(out=q_nat[:, 16:32, :], in_=q_rr[:, 16:32, :])

    # ---------------- constants / parameters ----------------
    eps_sb = consts.tile([P, 1], F32, tag="eps")
    nc.vector.memset(eps_sb, EPS)
    nbias_sb = consts.tile([P, 1], F32, tag="nbias")
    nc.vector.memset(nbias_sb, -2.5)

    # Pin the exp ACT table set immediately: the (walrus-inserted) ~1.3us
    # table load runs during the input DMAs instead of gating the first exp.
    pin_sb = consts.tile([1, 1], F32, tag="pin")
    nc.scalar.activation(pin_sb, eps_sb[0:1, :], ACTF.Exp)

    ident = consts.tile([P, P], F32, tag="ident")
    make_identity(nc, ident)
    ident_bf = consts.tile([P, P], BF16, tag="ident_bf")
    nc.vector.tensor_copy(ident_bf, ident)

    wdif_f = consts.tile([D, D], F32, tag="wdif_f")
    nc.sync.dma_start(out=wdif_f, in_=w_dif)
    wdif_sb = consts.tile([D, D], BF16, tag="wdif")
    nc.vector.tensor_copy(wdif_sb, wdif_f)

    w1_sb = consts.tile([D, H], F32, tag="w1")
    nc.sync.dma_start(out=w1_sb, in_=w1)
    gamma_sb = consts.tile([D, 1], F32, tag="gamma")
    nc.sync.dma_start(out=gamma_sb, in_=gamma[:, None])
    beta_sb = consts.tile([D, 1], F32, tag="beta")
    nc.sync.dma_start(out=beta_sb, in_=beta[:, None])

    # Fold LN gamma into W1 (h_hat * gamma @ W1 = h_hat @ (gamma[:,None]*W1));
    # beta's contribution lands in the bias: b1' = b1 + beta @ W1.
    w1p_sb = consts.tile([D, H], BF16, tag="w1p")
    nc.vector.tensor_scalar_mul(w1p_sb, w1_sb, gamma_sb)

    b1_sb = consts.tile([P, 2], F32, tag="b1")
    nc.sync.dma_start(out=b1_sb, in_=b1.rearrange("(a p) -> p a", p=P))

    w2f_sb = consts.tile([P, 2, D], F32, tag="w2f")
    nc.sync.dma_start(out=w2f_sb, in_=w2.rearrange("(a p) d -> p a d", p=P))
    w2_sb = consts.tile([P, 2, D], BF16, tag="w2")
    nc.vector.tensor_copy(w2_sb, w2f_sb)

    b2_bc = consts.tile([P, TPC, D], F32, tag="b2bc")
    _bcast_free(nc, b2_bc, b2)
    bdif_bc = consts.tile([P, TPC, D], F32, tag="bdifbc")
    _bcast_free(nc, bdif_bc, b_dif)

    ones_bf = consts.tile([1, D], BF16, tag="ones")
    nc.vector.memset(ones_bf, 1.0)


    # ---------------- bf16 copies + transposed layouts ----------------
    k_bf = big.tile([P, NT, D], BF16, tag="k_bf")
    q_bf = big.tile([P, NT, D], BF16, tag="q_bf")
    v_bf = big.tile([P, NT, D], BF16, tag="v_bf")

    qT = big.tile([P, N], BF16, tag="qT")   # rows 0-63 and 64-127 both hold q^T
    kT = big.tile([P, N], BF16, tag="kT")   # rows 0-63 and 64-127 both hold k^T
    vT = big.tile([D, N], BF16, tag="vT")

    b1p_sb = consts.tile([P, 2], F32, tag="b1p")

    # init-phase PSUM pool: k transposes + q chunk-0 transpose.
    # Closed before the pass-1 pools open so the banks get reused.
    # (b1p's matmuls are emitted in pass 2: they need the late-arriving
    # W1/beta DMAs and would stall the in-order tensor queue here.)
    with ExitStack() as sctx:
        ps_init = sctx.enter_context(tc.tile_pool(name="ps_init", bufs=2, space="PSUM"))
        # k transposes: 4 groups of 8 tiles (cast -> matmul-transpose -> evac,
        # pipelined per DMA piece), regular matmuls vs identity
        for gidx in range(4):
            nc.vector.tensor_copy(k_bf[:, gidx * 8:(gidx + 1) * 8, :],
                                  k_nat[:, gidx * 8:(gidx + 1) * 8, :])
            pt = ps_init.tile([D, 8 * P], F32, tag="ktr", name=f"ktr{gidx}")
            for s in range(8):
                t = gidx * 8 + s
                nc.tensor.matmul(pt[:, s * P:(s + 1) * P], k_bf[:, t, :], ident_bf,
                                 start=True, stop=True)
            nc.vector.tensor_copy(kT[0:D, gidx * 8 * P:(gidx + 1) * 8 * P], pt)
            # duplicate k^T into rows 64..127 (cheap bf16 DVE copy per group)
            nc.vector.tensor_copy(kT[D:P, gidx * 8 * P:(gidx + 1) * 8 * P],
                                  kT[0:D, gidx * 8 * P:(gidx + 1) * 8 * P])
        nc.vector.tensor_copy(q_bf[:, 0:TPC, :], q_nat[:, 0:TPC, :])

        # q chunk 0 transpose (4 tiles)
        pt = ps_init.tile([D, TPC * P], F32, tag="qtr0")
        for s in range(TPC):
            nc.tensor.matmul(pt[:, s * P:(s + 1) * P], q_bf[:, s, :], ident_bf,
                             start=True, stop=True)
        nc.vector.tensor_copy(qT[0:D, 0:CH], pt)
        nc.vector.tensor_copy(qT[D:P, 0:CH], qT[0:D, 0:CH])

    # V with an appended ones column: the PV matmul then also produces the
    # softmax denominators (row 64 of the accumulator).
    # DoubleRow fp8 layout: pairs of j-tiles interleaved on the ko axis,
    # inner stride padded to 80 bytes (16-aligned). Ones column -> denominators.
    v_aug = big.tile([P, NT // 2, 2, 80], FP8, tag="v_aug")
    with nc.allow_low_precision(reason="softmax-averaged fp8 PV"):
        for gidx in range(4):
            nc.vector.tensor_copy(v_aug[:, gidx * 4:(gidx + 1) * 4, :, 0:D],
                                  v_nat[:, gidx * 8:(gidx + 1) * 8, :])
    nc.vector.memset(v_aug[:, :, :, D:D + 1], 1.0)

    v1_nat = big.tile([P, NT, D], BF16, tag="v1_nat")
    T_sb = big.tile([D, D], F32, tag="T_sb")
    nc.vector.memset(T_sb, 0.0)
    pT = big.tile([P, 2 * RB, CH], FP8, tag="pT")   # rotating exp output

    # ---------------- pass 1: flash attention + dif_proj + T ----------------
    with ExitStack() as p1:
        # One 2-bank tile per in-flight QK region (bufs=RB rotation). A single
        # manually-sliced 6-bank tile gets whole-tile dependency tracking:
        # QK(g) then waits on exp(g-1) instead of exp(g-RB), serializing the
        # exp <-> QK pipeline.
        ps_qk_pool = p1.enter_context(tc.tile_pool(name="ps_qk", bufs=RB, space="PSUM"))
        ps_attn = p1.enter_context(tc.tile_pool(name="ps_attn", bufs=1, space="PSUM"))
        ps_tail_pool = p1.enter_context(tc.tile_pool(name="ps_tail", bufs=1, space="PSUM"))

        tail_ps = ps_tail_pool.tile([P, CH], F32, tag="tail")       # 1 bank

        qk_tiles = {}
        attn_tiles = {}
        chunk_state = {}

        def emit_qk(g):
            c, jt2 = divmod(g, JT2)
            st = ps_qk_pool.tile([P, 2 * CH], F32, tag="qk", name=f"qk{g}")
            qk_tiles[g] = st
            i0 = c * CH
            for s in range(2):
                jt = jt2 * 2 + s
                r0 = s * D
                nc.tensor.matmul(
                    st[:, s * CH:(s + 1) * CH],
                    kT[r0:r0 + D, jt * P:(jt + 1) * P],
                    qT[r0:r0 + D, i0:i0 + CH],
                    start=True, stop=True,
                    tile_position=(r0, 0),
                )

        def emit_exp(g):
            r = g % RB
            nc.scalar.activation(
                pT[:, 2 * r:2 * r + 2, :],
                qk_tiles.pop(g),
                ACTF.Exp, bias=nbias_sb, scale=SCALE,
            )

        def emit_pv(j):
            c, jt2 = divmod(j, JT2)
            if jt2 == 0:
                attn_tiles[c] = ps_attn.tile([D + 1, CH], F32, tag="attn",
                                             name=f"attn_{c}")
            m = j % RB
            nc.tensor.matmul(
                attn_tiles[c],
                v_aug[:, jt2, :, 0:D + 1],
                pT[:, 2 * m:2 * m + 2, :],
                start=(jt2 == 0), stop=(jt2 == JT2 - 1),
                perf_mode=mybir.MatmulPerfMode.DoubleRow,
            )

        def tail_a(c):
            # evacuate attn accumulator promptly: with a single-buffered attn
            # bank, PV(c+1) start waits on this read.
            attn_sb = work.tile([D + 1, CH], F32, tag="attn_sb")
            nc.vector.tensor_copy(attn_sb, attn_tiles.pop(c))
            chunk_state[c] = attn_sb

        def tail_b(c):
            attn_sb = chunk_state[c]
            recip_sb = work.tile([1, CH], F32, tag="recip")
            nc.vector.reciprocal(recip_sb, attn_sb[D:D + 1, :])
            i0 = c * CH
            recipb_ps = tail_ps[0:D, :]
            # bf16 broadcast matmul: an f32 one runs as a slow LOW/HIGH 2-pass
            recip_bf = work.tile([1, CH], BF16, tag="recipbf")
            with nc.allow_low_precision(reason="softmax denom tolerates bf16"):
                nc.vector.tensor_copy(recip_bf, recip_sb)
            nc.tensor.matmul(recipb_ps, ones_bf, recip_bf, start=True, stop=True)
            tmp = work.tile([D, CH], BF16, tag="tmp")
            diffT = work.tile([D, CH], BF16, tag="diffT")
            with nc.allow_low_precision(reason="dif branch tolerates bf16"):
                nc.vector.tensor_mul(tmp, attn_sb[0:D, :], recipb_ps)
                nc.vector.tensor_sub(diffT, vT[:, i0:i0 + CH], tmp)
            chunk_state[c] = diffT

        def tail_c(c):
            diffT = chunk_state.pop(c)
            v1_ps = tail_ps[:, 0:TPC * D]
            for s in range(TPC):
                nc.tensor.matmul(
                    v1_ps[:, s * D:(s + 1) * D],
                    diffT[:, s * P:(s + 1) * P],
                    wdif_sb,
                    start=True, stop=True,
                )
            with nc.allow_low_precision(reason="v1 tolerates bf16"):
                nc.vector.tensor_add(v1_nat[:, c * TPC:(c + 1) * TPC, :],
                                     v1_ps, bdif_bc)
            T_part = tail_ps[0:D, TPC * D:TPC * D + D]
            for s in range(TPC):
                t = c * TPC + s
                nc.tensor.matmul(
                    T_part,
                    k_bf[:, t, :],
                    v1_nat[:, t, :],
                    start=(s == 0), stop=(s == TPC - 1),
                )
            nc.vector.tensor_add(T_sb, T_sb, T_part)

        def tr_q_group(c):
            # build qT for chunk c (tiles 4c..4c+3) through the tail bank
            nc.vector.tensor_copy(q_bf[:, c * TPC:(c + 1) * TPC, :],
                                  q_nat[:, c * TPC:(c + 1) * TPC, :])
            ptw = tail_ps[0:D, :]  # [64, 512] f32
            for s in range(TPC):
                t = c * TPC + s
                nc.tensor.matmul(ptw[:, s * P:(s + 1) * P], q_bf[:, t, :], ident_bf,
                                 start=True, stop=True)
            nc.vector.tensor_copy(qT[0:D, c * CH:(c + 1) * CH], ptw)
            nc.vector.tensor_copy(qT[D:P, c * CH:(c + 1) * CH],
                                  qT[0:D, c * CH:(c + 1) * CH])

        def tr_v_group(c):
            # build vT for chunk c (tiles 4c..4c+3); bf16 matmuls (f32 would
            # run as slow LOW/HIGH 2-pass)
            with nc.allow_low_precision(reason="dif branch tolerates bf16"):
                nc.vector.tensor_copy(v_bf[:, c * TPC:(c + 1) * TPC, :],
                                      v_nat[:, c * TPC:(c + 1) * TPC, :])
            ptw = tail_ps[0:D, :]
            for s in range(TPC):
                t = c * TPC + s
                nc.tensor.matmul(ptw[:, s * P:(s + 1) * P], v_bf[:, t, :], ident_bf,
                                 start=True, stop=True)
            nc.vector.tensor_copy(vT[:, c * CH:(c + 1) * CH], ptw)

        # pipelined emission over 128 global iterations
        for g in range(G):
            c, jt2 = divmod(g, JT2)
            emit_qk(g)
            emit_exp(g)
            if g >= 2:
                emit_pv(g - 2)
            # per-chunk tails on the previous chunk, staggered
            if c >= 1:
                if jt2 == 3:
                    tail_a(c - 1)
                elif jt2 == 6:
                    tail_b(c - 1)
                elif jt2 == 9:
                    tail_c(c - 1)
            # deferred transposes through the tail bank
            if jt2 == 12 and c + 1 < NCH:
                tr_q_group(c + 1)
            if jt2 == 14 and c < NCH:
                tr_v_group(c)
        emit_pv(G - 2)
        emit_pv(G - 1)
        tail_a(NCH - 1)
        tail_b(NCH - 1)
        tail_c(NCH - 1)

    # T picks up the deferred 1/sqrt(D) score scaling; dual-packed bf16 copy
    # for the pass-2 v_new matmuls (tile_position halves need partition-
    # matched rhs operands).
    T_bf = consts.tile([D, D], BF16, tag="T_bf")
    with nc.allow_low_precision(reason="v_new correction tolerates bf16"):
        nc.vector.tensor_scalar_mul(T_bf, T_sb, SCALE)

    # ---------------- pass 2: v_new, LN, MLP, residual ----------------
    # Phase A: v_new + LN stats for ALL chunks, then a single batched
    # sqrt+reciprocal (one ACT table set load instead of one per chunk),
    # then the LN applies (Identity: present in every table set).
    # Phase B: hT / z1 / gelu / mlp, gelu set loaded exactly once.
    vn_all = big.tile([P, NT, D], F32, tag="vn_all")
    h_all = big.tile([P, NT, D], BF16, tag="h_all")
    sq_buf = big.tile([P, NT, D], F32, tag="sq_buf")   # squares, then LN scratch
    vb_all = big.tile([P, NT, D], F32, tag="vb_all")   # v_new + b2 for residual
    sum_t = big.tile([P, NT], F32, tag="sum_t")
    sq_t = big.tile([P, NT], F32, tag="sq_t")
    mu_t = big.tile([P, NT], F32, tag="mu_t")       # holds -mean
    var_t = big.tile([P, NT], F32, tag="var_t")
    rstd_t = big.tile([P, NT], F32, tag="rstd_t")
    rr_t = big.tile([P, NT], F32, tag="rr_t")
    nmr_t = big.tile([P, NT], F32, tag="nmr_t")     # -mean * rstd

    def _bc(tile_ap, reps):
        """Append a 0-stride broadcast dim of `reps` to an AP."""
        return bass.AP(tensor=tile_ap.tensor, offset=tile_ap.offset,
                       ap=list(tile_ap.ap) + [[0, reps]])

    with ExitStack() as pa:
        ps_a = pa.enter_context(tc.tile_pool(name="ps_a", bufs=2, space="PSUM"))
        paw = pa.enter_context(tc.tile_pool(name="paw", bufs=2))
        # deferred b1' = b1 + beta @ W1 (needs late param DMAs)
        for a in range(2):
            bw = ps_a.tile([P, 1], F32, tag="bw")
            nc.tensor.matmul(
                bw, w1_sb[:, a * P:(a + 1) * P], beta_sb, start=True, stop=True
            )
            nc.vector.tensor_add(b1p_sb[:, a:a + 1], bw, b1_sb[:, a:a + 1])

        for cc in range(NCH // 2):   # two chunks per PSUM tile
            vn_ps = ps_a.tile([P, 2 * TPC * D], F32, tag="vn")
            for s in range(2 * TPC):
                t = cc * 2 * TPC + s
                nc.tensor.matmul(
                    vn_ps[:, s * D:(s + 1) * D],
                    qT[0:D, t * P:(t + 1) * P],
                    T_bf,
                    start=True, stop=True,
                )
            sl = slice(cc * 2 * TPC, (cc + 1) * 2 * TPC)
            nc.vector.tensor_add(vn_all[:, sl, :], vn_ps, q_nat[:, sl, :])

        # batched LN stats: one square (ACT), two big reduces (DVE)
        nc.scalar.square(sq_buf, vn_all)
        nc.vector.tensor_reduce(sum_t, vn_all, axis=mybir.AxisListType.X,
                                op=ALU.add)
        nc.vector.tensor_reduce(sq_t, sq_buf, axis=mybir.AxisListType.X,
                                op=ALU.add)
        nc.vector.tensor_scalar_mul(mu_t, sum_t, -1.0 / D)
        nc.vector.tensor_scalar_mul(var_t, sq_t, 1.0 / D)
        musq = paw.tile([P, NT], F32, tag="musq")
        nc.vector.tensor_mul(musq, mu_t, mu_t)
        nc.vector.tensor_sub(var_t, var_t, musq)
        nc.scalar.activation(rstd_t, var_t, ACTF.Sqrt, bias=eps_sb)
        nc.vector.reciprocal(rr_t, rstd_t)
        nc.vector.tensor_mul(nmr_t, mu_t, rr_t)
        # residual + b2, folded once so s5 does a single add per chunk
        b2row = b2_bc[:, 0, :]
        nc.vector.tensor_tensor(
            out=vb_all, in0=vn_all,
            in1=bass.AP(tensor=b2row.tensor, offset=b2row.offset,
                        ap=[b2row.ap[0], [0, NT]] + [b2row.ap[1]]),
            op=ALU.add,
        )
        # LN apply h = (v_new - mu) * rstd via 0-stride broadcast APs,
        # in halves so phase B can start on the first chunks early.
        HNT = NT // 2
        for hh in range(2):
            sl = slice(hh * HNT, (hh + 1) * HNT)
            nc.vector.tensor_tensor(
                out=sq_buf[:, sl, :], in0=vn_all[:, sl, :],
                in1=_bc(rr_t[:, sl], D), op=ALU.mult,
            )
            with nc.allow_low_precision(reason="LN output feeds bf16 MLP"):
                nc.vector.tensor_tensor(
                    out=h_all[:, sl, :], in0=sq_buf[:, sl, :],
                    in1=_bc(nmr_t[:, sl], D), op=ALU.add,
                )

    with ExitStack() as p2:
        ps_ht = p2.enter_context(tc.tile_pool(name="ps_ht", bufs=2, space="PSUM"))
        ps_z1 = p2.enter_context(tc.tile_pool(name="ps_z1", bufs=2, space="PSUM"))
        ps_mlp = p2.enter_context(tc.tile_pool(name="ps_mlp", bufs=2, space="PSUM"))
        p2w = p2.enter_context(tc.tile_pool(name="p2w", bufs=4))

        state = {}

        def s3(c):
            # h^T via regular matmuls vs identity, then the MLP up-projection
            hT_ps = ps_ht.tile([D, CH], F32, tag="hT")
            for s in range(TPC):
                t = c * TPC + s
                nc.tensor.matmul(hT_ps[:, s * P:(s + 1) * P], h_all[:, t, :],
                                 ident_bf, start=True, stop=True)
            hT = p2w.tile([D, CH], BF16, tag="hTsb")
            nc.vector.tensor_copy(hT, hT_ps)
            z1_ps = ps_z1.tile([P, 2 * CH], F32, tag="z1")
            for a in range(2):
                nc.tensor.matmul(
                    z1_ps[:, a * CH:(a + 1) * CH],
                    w1p_sb[:, a * P:(a + 1) * P],
                    hT,
                    start=True, stop=True,
                )
            state[c] = z1_ps

        def s5(c):
            z1_ps = state.pop(c)
            g1 = p2w.tile([P, 2, CH], BF16, tag="g1")
            for a in range(2):
                nc.scalar.activation(
                    g1[:, a, :], z1_ps[:, a * CH:(a + 1) * CH],
                    ACTF.Gelu, bias=b1p_sb[:, a:a + 1],
                )
            mlp_ps = ps_mlp.tile([P, TPC * D], F32, tag="mlp")
            for s in range(TPC):
                for a in range(2):
                    nc.tensor.matmul(
                        mlp_ps[:, s * D:(s + 1) * D],
                        g1[:, a, s * P:(s + 1) * P],
                        w2_sb[:, a, :],
                        start=(a == 0), stop=(a == 1),
                    )
            sl = slice(c * TPC, (c + 1) * TPC)
            o2 = p2w.tile([P, TPC, D], F32, tag="o2")
            nc.vector.tensor_add(o2, mlp_ps, vb_all[:, sl, :])
            nc.sync.dma_start(
                out=out.rearrange("(p t) d -> p t d", t=NT)[:, sl, :],
                in_=o2,
            )

        for step in range(NCH + 1):
            if step < NCH:
                s3(step)
            if 0 <= step - 1 < NCH:
                s5(step - 1)


_NC_CACHE = None


def _get_nc():
    global _NC_CACHE
    if _NC_CACHE is None:
        _NC_CACHE = build_nc()
    return _NC_CACHE


def kernel(**inputs) -> np.ndarray:
    nc = _get_nc()
    per_batch = {"q", "k", "v"}
    in_maps = []
    for b in range(B):
        m = {}
        for name, arr in inputs.items():
            arr = np.asarray(arr)
            m[name] = np.ascontiguousarray(arr[b] if name in per_batch else arr)
        in_maps.append(m)
    res = run_bass_kernel_spmd(nc, in_maps, core_ids=list(range(B)))
    return np.stack([res.results[i]["out"] for i in range(B)], axis=0)

